# revision 5
# baseline (speedup 1.0000x reference)
"""Trainium2 Bass kernel for a 2-layer GRU phoneme decoder (teacher forcing).

Model: B=512, T=128, H=1024, L=2, V=128.
  tokens = [inp[:,0], target[:,1:]]          (B, T)
  per step t: x = emb[tokens[:,t]]; 2 GRU layers; logits = h1 @ emb.T

Strategy (per the data-parallel sharding hint):
  - Shard batch B=512 across 8 NeuronCores (64 rows each); weights replicated.
  - "Transposed world" layout: feature dims live on SBUF partitions, the
    64-row batch lives on the free axis. All matmuls are weight-stationary:
        out[gates_tile(128), batch(64)] += W_tile(128K,128M).T @ hT(128K,64)
  - Layer-0 input matmul is eliminated: gx0 = onehot(token) @ (emb @ W_ih0.T
    + b_ih0 + b_hh0[rz]) via a 128-row table (vocab on partitions, K=128).
  - Biases folded with K=1 rank-1 matmuls into the PSUM accumulation chains.
  - Gate math fp32 on full-width (128, 512) slabs; h state fp32, cast to bf16
    for the next matmul.
  - All weights stay resident in SBUF for the whole 128-step recurrence.
"""

import os
import sys

import numpy as np
import ml_dtypes

sys.path.insert(0, "/opt/trn_rl_repo")

VOCAB, H, L, B, T = 128, 1024, 2, 512, 128
NCORES = 8
BC = B // NCORES  # 64 batch rows per core
GH = 3 * H  # 3072 gates
MT = GH // 128  # 24 gate M-tiles
KT = H // 128  # 8 contraction K-tiles
BF = ml_dtypes.bfloat16

# bias vector layout (single row, bf16): [b_hh0_n | b_rz1 | b_ih1_n | b_hh1_n]
OFF_NH0 = 0
OFF_RZ1 = 1024
OFF_NX1 = 1024 + 2048
OFF_NH1 = 1024 + 2048 + 1024
BIAS_LEN = 1024 + 2048 + 1024 + 1024


def _prep_host(inp, target, hidden_state, embedding, W_ih, W_hh, b_ih, b_hh):
    """Pack full inputs into per-core in_maps (all hardware layouts)."""
    inp = np.asarray(inp)
    target = np.asarray(target)
    hidden_state = np.asarray(hidden_state, dtype=np.float32)
    embedding = np.asarray(embedding, dtype=np.float32)
    W_ih = np.asarray(W_ih, dtype=np.float32)
    W_hh = np.asarray(W_hh, dtype=np.float32)
    b_ih = np.asarray(b_ih, dtype=np.float32)
    b_hh = np.asarray(b_hh, dtype=np.float32)

    tokens = np.concatenate([inp[:, :1], target[:, 1:]], axis=1).astype(np.int64)

    def wtiles(W):  # (3072, 1024) -> (128, KT*MT*128) lhsT tiles [p][k,m,c]
        W4 = W.reshape(MT, 128, KT, 128)  # [m, c, k, p]
        return np.ascontiguousarray(
            W4.transpose(3, 2, 0, 1).reshape(128, KT * MT * 128)
        ).astype(BF)

    whh0 = wtiles(W_hh[0])
    wih1 = wtiles(W_ih[1])
    whh1 = wtiles(W_hh[1])

    # layer-0 input table: includes b_ih0 (all gates) and b_hh0 (r,z only)
    tab = embedding @ W_ih[0].T + b_ih[0]
    tab[:, : 2 * H] += b_hh[0][: 2 * H]
    gxtab = np.ascontiguousarray(tab).astype(BF)  # (128, 3072) [v][m*128+c]

    # logits lhsT tiles: [p][k*128+v] = emb[v, k*128+p]
    embt = np.ascontiguousarray(
        embedding.reshape(VOCAB, KT, 128).transpose(2, 1, 0).reshape(128, KT * VOCAB)
    ).astype(BF)

    bias = np.zeros((1, BIAS_LEN), np.float32)
    bias[0, OFF_NH0:OFF_NH0 + H] = b_hh[0][2 * H:]
    bias[0, OFF_RZ1:OFF_RZ1 + 2 * H] = b_ih[1][: 2 * H] + b_hh[1][: 2 * H]
    bias[0, OFF_NX1:OFF_NX1 + H] = b_ih[1][2 * H:]
    bias[0, OFF_NH1:OFF_NH1 + H] = b_hh[1][2 * H:]
    bias = bias.astype(BF)
    ones = np.ones((1, BC), BF)

    in_maps = []
    for c in range(NCORES):
        sl = slice(c * BC, (c + 1) * BC)
        tk = tokens[sl]  # (BC, T)
        oh = (np.arange(VOCAB)[:, None, None] == tk.T[None, :, :]).astype(BF)
        oh = np.ascontiguousarray(oh.reshape(VOCAB, T * BC))  # [v][t*BC+b]

        def htile(hl):  # (BC, 1024) -> (128, KT*BC) f32  [p][k*BC+b]
            return np.ascontiguousarray(
                hl.reshape(BC, KT, 128).transpose(2, 1, 0).reshape(128, KT * BC)
            ).astype(np.float32)

        in_maps.append(
            {
                "whh0": whh0,
                "wih1": wih1,
                "whh1": whh1,
                "gxtab": gxtab,
                "embt": embt,
                "bias": bias,
                "ones": ones,
                "oh": oh,
                "h0": htile(hidden_state[0, sl]),
                "h1": htile(hidden_state[1, sl]),
            }
        )
    return in_maps


def _build_program(t_steps, unroll):
    import concourse.bass as bass
    import concourse.mybir as mybir
    import concourse.tile as tile
    from concourse import bacc
    from contextlib import ExitStack

    f32 = mybir.dt.float32
    bf16 = mybir.dt.bfloat16
    AF = mybir.ActivationFunctionType

    nc = bacc.Bacc("TRN2", target_bir_lowering=False, debug=False)

    p_whh0 = nc.declare_dram_parameter("whh0", [128, KT * MT * 128], bf16, isOutput=False)
    p_wih1 = nc.declare_dram_parameter("wih1", [128, KT * MT * 128], bf16, isOutput=False)
    p_whh1 = nc.declare_dram_parameter("whh1", [128, KT * MT * 128], bf16, isOutput=False)
    p_gxtab = nc.declare_dram_parameter("gxtab", [128, MT * 128], bf16, isOutput=False)
    p_embt = nc.declare_dram_parameter("embt", [128, KT * VOCAB], bf16, isOutput=False)
    p_bias = nc.declare_dram_parameter("bias", [1, BIAS_LEN], bf16, isOutput=False)
    p_ones = nc.declare_dram_parameter("ones", [1, BC], bf16, isOutput=False)
    p_oh = nc.declare_dram_parameter("oh", [VOCAB, T * BC], bf16, isOutput=False)
    p_h0 = nc.declare_dram_parameter("h0", [128, KT * BC], f32, isOutput=False)
    p_h1 = nc.declare_dram_parameter("h1", [128, KT * BC], f32, isOutput=False)
    p_out = nc.declare_dram_parameter("out", [t_steps * VOCAB, BC], f32, isOutput=True)

    with ExitStack() as ctx:
        tc = ctx.enter_context(tile.TileContext(nc))
        wpool = ctx.enter_context(tc.tile_pool(name="w", bufs=1))
        state = ctx.enter_context(tc.tile_pool(name="state", bufs=1))
        gates = ctx.enter_context(tc.tile_pool(name="gates", bufs=1))
        stage = ctx.enter_context(tc.tile_pool(name="stage", bufs=3))
        psum = ctx.enter_context(tc.tile_pool(name="psum", bufs=2, space="PSUM"))

        def load(pool, param, shape, dtype, tag):
            t_ = pool.tile(shape, dtype, tag=tag)
            nc.sync.dma_start(t_[:], param.ap())
            return t_

        s_whh0 = load(wpool, p_whh0, [128, KT * MT * 128], bf16, "whh0")
        s_wih1 = load(wpool, p_wih1, [128, KT * MT * 128], bf16, "wih1")
        s_whh1 = load(wpool, p_whh1, [128, KT * MT * 128], bf16, "whh1")
        s_gxtab = load(wpool, p_gxtab, [128, MT * 128], bf16, "gxtab")
        s_embt = load(wpool, p_embt, [128, KT * VOCAB], bf16, "embt")
        s_bias = load(wpool, p_bias, [1, BIAS_LEN], bf16, "bias")
        s_ones = load(wpool, p_ones, [1, BC], bf16, "ones")
        s_oh = load(wpool, p_oh, [VOCAB, T * BC], bf16, "oh_all")
        s_h0f = load(state, p_h0, [128, KT * BC], f32, "h0f")
        s_h1f = load(state, p_h1, [128, KT * BC], f32, "h1f")
        s_h0b = state.tile([128, KT * BC], bf16, tag="h0b")
        s_h1b = state.tile([128, KT * BC], bf16, tag="h1b")
        nc.vector.tensor_copy(s_h0b[:], s_h0f[:])
        nc.vector.tensor_copy(s_h1b[:], s_h1f[:])

        def wsl(s, k, m):  # weight lhsT tile (128, 128)
            o = (k * MT + m) * 128
            return s[:, o:o + 128]

        def hsl(s, k):  # h rhs tile (128, BC)
            return s[:, k * BC:(k + 1) * BC]

        def bank_slice(p, j):
            return p[:, j * BC:(j + 1) * BC]

        def gate_math(pr, pz, pnx, pnh, hf, hb):
            r_s = gates.tile([128, 8 * BC], f32, tag="rs")
            nc.scalar.activation(r_s[:], pr[:], AF.Sigmoid)
            z_s = gates.tile([128, 8 * BC], f32, tag="zs")
            nc.scalar.activation(z_s[:], pz[:], AF.Sigmoid)
            t1 = gates.tile([128, 8 * BC], f32, tag="tA")
            nc.vector.tensor_mul(t1[:], r_s[:], pnh[:])
            t2 = gates.tile([128, 8 * BC], f32, tag="tB")
            nc.vector.tensor_add(t2[:], pnx[:], t1[:])
            n_s = gates.tile([128, 8 * BC], f32, tag="ns")
            nc.scalar.activation(n_s[:], t2[:], AF.Tanh)
            d = gates.tile([128, 8 * BC], f32, tag="tA")
            nc.vector.tensor_sub(d[:], hf[:], n_s[:])
            m_ = gates.tile([128, 8 * BC], f32, tag="tB")
            nc.vector.tensor_mul(m_[:], z_s[:], d[:])
            nc.vector.tensor_add(hf[:], n_s[:], m_[:])
            nc.vector.tensor_copy(hb[:], hf[:])

        def bias_mm(pbank, j, boff, start):
            nc.tensor.matmul(
                bank_slice(pbank, j),
                s_bias[0:1, boff + j * 128: boff + (j + 1) * 128],
                s_ones[0:1, :],
                start=start,
                stop=False,
            )

        def step(t):
            oh_t = stage.tile([128, BC], bf16, tag="oh")
            nc.vector.tensor_copy(oh_t[:], s_oh[:, bass.ds(t * BC, BC)])

            # ---- layer 0 ----
            pr = psum.tile([128, 8 * BC], f32, tag="r")
            pz = psum.tile([128, 8 * BC], f32, tag="z")
            pnx = psum.tile([128, 8 * BC], f32, tag="nx")
            pnh = psum.tile([128, 8 * BC], f32, tag="nh")
            for j in range(8):  # r: table + gh chains
                nc.tensor.matmul(bank_slice(pr, j), s_gxtab[:, j * 128:(j + 1) * 128],
                                 oh_t[:], start=(j == 0), stop=False)
                for k in range(KT):
                    nc.tensor.matmul(bank_slice(pr, j), wsl(s_whh0, k, j),
                                     hsl(s_h0b, k), start=False,
                                     stop=(j == 7 and k == KT - 1))
            for j in range(8):  # z
                m = 8 + j
                nc.tensor.matmul(bank_slice(pz, j), s_gxtab[:, m * 128:(m + 1) * 128],
                                 oh_t[:], start=(j == 0), stop=False)
                for k in range(KT):
                    nc.tensor.matmul(bank_slice(pz, j), wsl(s_whh0, k, m),
                                     hsl(s_h0b, k), start=False,
                                     stop=(j == 7 and k == KT - 1))
            for j in range(8):  # nx: table only (b_ih0 baked in)
                m = 16 + j
                nc.tensor.matmul(bank_slice(pnx, j), s_gxtab[:, m * 128:(m + 1) * 128],
                                 oh_t[:], start=(j == 0), stop=(j == 7))
            for j in range(8):  # nh: bias + gh
                m = 16 + j
                bias_mm(pnh, j, OFF_NH0, start=(j == 0))
                for k in range(KT):
                    nc.tensor.matmul(bank_slice(pnh, j), wsl(s_whh0, k, m),
                                     hsl(s_h0b, k), start=False,
                                     stop=(j == 7 and k == KT - 1))
            gate_math(pr, pz, pnx, pnh, s_h0f, s_h0b)

            # ---- layer 1 ----
            pr1 = psum.tile([128, 8 * BC], f32, tag="r")
            pz1 = psum.tile([128, 8 * BC], f32, tag="z")
            pnx1 = psum.tile([128, 8 * BC], f32, tag="nx")
            pnh1 = psum.tile([128, 8 * BC], f32, tag="nh")
            for j in range(8):  # r = bias + gx(h0) + gh(h1)
                bias_mm(pr1, j, OFF_RZ1, start=(j == 0))
                for k in range(KT):
                    nc.tensor.matmul(bank_slice(pr1, j), wsl(s_wih1, k, j),
                                     hsl(s_h0b, k), start=False, stop=False)
                for k in range(KT):
                    nc.tensor.matmul(bank_slice(pr1, j), wsl(s_whh1, k, j),
                                     hsl(s_h1b, k), start=False,
                                     stop=(j == 7 and k == KT - 1))
            for j in range(8):  # z
                m = 8 + j
                bias_mm(pz1, j, OFF_RZ1 + 1024, start=(j == 0))
                for k in range(KT):
                    nc.tensor.matmul(bank_slice(pz1, j), wsl(s_wih1, k, m),
                                     hsl(s_h0b, k), start=False, stop=False)
                for k in range(KT):
                    nc.tensor.matmul(bank_slice(pz1, j), wsl(s_whh1, k, m),
                                     hsl(s_h1b, k), start=False,
                                     stop=(j == 7 and k == KT - 1))
            for j in range(8):  # nx
                m = 16 + j
                bias_mm(pnx1, j, OFF_NX1, start=(j == 0))
                for k in range(KT):
                    nc.tensor.matmul(bank_slice(pnx1, j), wsl(s_wih1, k, m),
                                     hsl(s_h0b, k), start=False,
                                     stop=(j == 7 and k == KT - 1))
            for j in range(8):  # nh
                m = 16 + j
                bias_mm(pnh1, j, OFF_NH1, start=(j == 0))
                for k in range(KT):
                    nc.tensor.matmul(bank_slice(pnh1, j), wsl(s_whh1, k, m),
                                     hsl(s_h1b, k), start=False,
                                     stop=(j == 7 and k == KT - 1))
            gate_math(pr1, pz1, pnx1, pnh1, s_h1f, s_h1b)

            # ---- logits ----
            plg = psum.tile([128, BC], f32, tag="nx")
            for k in range(KT):
                nc.tensor.matmul(plg[:], s_embt[:, k * VOCAB:(k + 1) * VOCAB],
                                 hsl(s_h1b, k), start=(k == 0), stop=(k == KT - 1))
            lsb = stage.tile([128, BC], f32, tag="lg")
            nc.scalar.copy(lsb[:], plg[:])
            nc.sync.dma_start(p_out.ap()[bass.ds(t * VOCAB, VOCAB), :], lsb[:])

        if unroll >= t_steps:
            for t in range(t_steps):
                step(t)
        else:
            import concourse.mybir as mybir_
            with tc.For_i(0, t_steps, unroll,
                          hint_engines=(mybir_.EngineType.PE,)) as tv:
                for u in range(unroll):
                    step(tv + u)

    nc.compile()
    return nc


_PROGRAM_CACHE = {}


def _get_program(t_steps, unroll):
    key = (t_steps, unroll)
    if key not in _PROGRAM_CACHE:
        _PROGRAM_CACHE[key] = _build_program(t_steps, unroll)
    return _PROGRAM_CACHE[key]


def run(inputs, t_steps=T, unroll=2, trace=False):
    """Run on 8 cores. Returns (full_output (B,T,V) f32, BassKernelResults)."""
    from concourse.bass_utils import run_bass_kernel_spmd

    in_maps = _prep_host(**inputs)
    if t_steps != T:
        for m in in_maps:
            pass  # oh stays full-size; kernel only reads first t_steps slices
    nc = _get_program(t_steps, unroll)
    res = run_bass_kernel_spmd(nc, in_maps, core_ids=list(range(NCORES)),
                               trace=trace)
    out = np.empty((B, t_steps, VOCAB), np.float32)
    for c in range(NCORES):
        arr = np.asarray(res.results[c]["out"]).reshape(t_steps, VOCAB, BC)
        out[c * BC:(c + 1) * BC] = arr.transpose(2, 0, 1)
    return out, res


def kernel(**inputs) -> np.ndarray:
    out, _ = run(inputs, t_steps=T, unroll=2, trace=False)
    return out


def bench(inputs, t_steps=T, unroll=2, iters=3):
    """Build the sharded jit once, run repeatedly, return (out, times_sec)."""
    import time

    import jax
    import jax.numpy as jnp
    from jax.sharding import Mesh, PartitionSpec
    from jax.experimental.shard_map import shard_map
    import concourse.mybir as mybir
    from concourse import bass2jax
    from concourse.bass2jax import _bass_exec_p, partition_id_tensor

    bass2jax.install_neuronx_cc_hook()
    in_maps = _prep_host(**inputs)
    nc = _get_program(t_steps, unroll)

    partition_name = nc.partition_id_tensor.name if nc.partition_id_tensor else None
    in_names, out_names, out_avals, zero_outs = [], [], [], []
    for alloc in nc.m.functions[0].allocations:
        if not isinstance(alloc, mybir.MemoryLocationSet):
            continue
        name = alloc.memorylocations[0].name
        if alloc.kind == "ExternalInput":
            if name != partition_name:
                in_names.append(name)
        elif alloc.kind == "ExternalOutput":
            out_names.append(name)
            shape = tuple(alloc.tensor_shape)
            dtype = mybir.dt.np(alloc.dtype)
            out_avals.append(jax.core.ShapedArray(shape, dtype))
            zero_outs.append(np.zeros(shape, dtype))
    n_params = len(in_names)
    n_outs = len(out_avals)
    all_in_names = list(in_names) + list(out_names)
    if partition_name is not None:
        all_in_names.append(partition_name)
    donate = tuple(range(n_params, n_params + n_outs))

    def _body(*args):
        operands = list(args)
        if partition_name is not None:
            operands.append(partition_id_tensor())
        outs = _bass_exec_p.bind(
            *operands,
            out_avals=tuple(out_avals),
            in_names=tuple(all_in_names),
            out_names=tuple(out_names),
            lowering_input_output_aliases=(),
            sim_require_finite=True,
            sim_require_nnan=True,
            nc=nc,
        )
        return tuple(outs)

    devices = jax.devices()[:NCORES]
    mesh = Mesh(np.asarray(devices), ("core",))
    in_specs = (PartitionSpec("core"),) * (n_params + n_outs)
    out_specs = (PartitionSpec("core"),) * len(out_names)
    sharded = jax.jit(
        shard_map(_body, mesh=mesh, in_specs=in_specs, out_specs=out_specs,
                  check_rep=False),
        donate_argnums=donate, keep_unused=True,
    )
    concat_in = [
        np.concatenate([np.asarray(in_maps[c][nm]) for c in range(NCORES)], axis=0)
        for nm in in_names
    ]
    sharding = jax.sharding.NamedSharding(mesh, PartitionSpec("core"))
    dev_in = [jax.device_put(a, sharding) for a in concat_in]

    def zeros():
        return [jax.device_put(
            np.zeros((NCORES * z.shape[0], *z.shape[1:]), z.dtype), sharding)
            for z in zero_outs]

    out_arrs = sharded(*dev_in, *zeros())
    jax.block_until_ready(out_arrs)
    times = []
    for _ in range(iters):
        zs = zeros()
        jax.block_until_ready(zs)
        t0 = time.perf_counter()
        out_arrs2 = sharded(*dev_in, *zeros())
        jax.block_until_ready(out_arrs2)
        times.append(time.perf_counter() - t0)
        out_arrs = out_arrs2

    out = np.empty((B, t_steps, VOCAB), np.float32)
    full = np.asarray(out_arrs[0]).reshape(NCORES, t_steps, VOCAB, BC)
    for c in range(NCORES):
        out[c * BC:(c + 1) * BC] = full[c].transpose(2, 0, 1)
    return out, times


# revision 6
# speedup vs baseline: 2.3096x; 2.3096x over previous
"""Trainium2 Bass kernel for a 2-layer GRU phoneme decoder (teacher forcing).

Model: B=512, T=128, H=1024, L=2, V=128.
  tokens = [inp[:,0], target[:,1:]]          (B, T)
  per step t: x = emb[tokens[:,t]]; 2 GRU layers; logits = h1 @ emb.T

Strategy (per the data-parallel sharding hint):
  - Shard batch B=512 across 8 NeuronCores (64 rows each); weights replicated.
  - "Transposed world" layout: feature dims live on SBUF partitions, the
    64-row batch lives on the free axis. All matmuls are weight-stationary:
        out[gates_tile(128), batch(64)] += W_tile(128K,128M).T @ hT(128K,64)
  - Layer-0 input matmul is eliminated: gx0 = onehot(token) @ (emb @ W_ih0.T
    + b_ih0 + b_hh0[rz]) via a 128-row table (vocab on partitions, K=128).
  - Biases folded with K=1 rank-1 matmuls into the PSUM accumulation chains.
  - Gate math fp32 on full-width (128, 512) slabs; h state fp32, cast to bf16
    for the next matmul.
  - All weights stay resident in SBUF for the whole 128-step recurrence.
"""

import os
import sys

import numpy as np
import ml_dtypes

sys.path.insert(0, "/opt/trn_rl_repo")

VOCAB, H, L, B, T = 128, 1024, 2, 512, 128
NCORES = 8
BC = B // NCORES  # 64 batch rows per core
GH = 3 * H  # 3072 gates
MT = GH // 128  # 24 gate M-tiles
KT = H // 128  # 8 contraction K-tiles
BF = ml_dtypes.bfloat16

# bias vector layout (single row, bf16): [b_hh0_n | b_rz1 | b_ih1_n | b_hh1_n]
OFF_NH0 = 0
OFF_RZ1 = 1024
OFF_NX1 = 1024 + 2048
OFF_NH1 = 1024 + 2048 + 1024
BIAS_LEN = 1024 + 2048 + 1024 + 1024


def _prep_host(inp, target, hidden_state, embedding, W_ih, W_hh, b_ih, b_hh):
    """Pack full inputs into per-core in_maps (all hardware layouts)."""
    inp = np.asarray(inp)
    target = np.asarray(target)
    hidden_state = np.asarray(hidden_state, dtype=np.float32)
    embedding = np.asarray(embedding, dtype=np.float32)
    W_ih = np.asarray(W_ih, dtype=np.float32)
    W_hh = np.asarray(W_hh, dtype=np.float32)
    b_ih = np.asarray(b_ih, dtype=np.float32)
    b_hh = np.asarray(b_hh, dtype=np.float32)

    tokens = np.concatenate([inp[:, :1], target[:, 1:]], axis=1).astype(np.int64)

    def wtiles(W):  # (3072, 1024) -> (128, KT*MT*128) lhsT tiles [p][k,m,c]
        W4 = W.reshape(MT, 128, KT, 128)  # [m, c, k, p]
        return np.ascontiguousarray(
            W4.transpose(3, 2, 0, 1).reshape(128, KT * MT * 128)
        ).astype(BF)

    whh0 = wtiles(W_hh[0])
    wih1 = wtiles(W_ih[1])
    whh1 = wtiles(W_hh[1])

    # layer-0 input table: includes b_ih0 (all gates) and b_hh0 (r,z only)
    tab = embedding @ W_ih[0].T + b_ih[0]
    tab[:, : 2 * H] += b_hh[0][: 2 * H]
    gxtab = np.ascontiguousarray(tab).astype(BF)  # (128, 3072) [v][m*128+c]

    # logits lhsT tiles: [p][k*128+v] = emb[v, k*128+p]
    embt = np.ascontiguousarray(
        embedding.reshape(VOCAB, KT, 128).transpose(2, 1, 0).reshape(128, KT * VOCAB)
    ).astype(BF)

    bias = np.zeros((1, BIAS_LEN), np.float32)
    bias[0, OFF_NH0:OFF_NH0 + H] = b_hh[0][2 * H:]
    bias[0, OFF_RZ1:OFF_RZ1 + 2 * H] = b_ih[1][: 2 * H] + b_hh[1][: 2 * H]
    bias[0, OFF_NX1:OFF_NX1 + H] = b_ih[1][2 * H:]
    bias[0, OFF_NH1:OFF_NH1 + H] = b_hh[1][2 * H:]
    bias = bias.astype(BF)
    ones = np.ones((1, BC), BF)

    in_maps = []
    for c in range(NCORES):
        sl = slice(c * BC, (c + 1) * BC)
        tk = tokens[sl]  # (BC, T)
        oh = (np.arange(VOCAB)[:, None, None] == tk.T[None, :, :]).astype(BF)
        oh = np.ascontiguousarray(oh.reshape(VOCAB, T * BC))  # [v][t*BC+b]

        def htile(hl):  # (BC, 1024) -> (128, KT*BC) f32  [p][k*BC+b]
            return np.ascontiguousarray(
                hl.reshape(BC, KT, 128).transpose(2, 1, 0).reshape(128, KT * BC)
            ).astype(np.float32)

        in_maps.append(
            {
                "whh0": whh0,
                "wih1": wih1,
                "whh1": whh1,
                "gxtab": gxtab,
                "embt": embt,
                "bias": bias,
                "ones": ones,
                "oh": oh,
                "h0": htile(hidden_state[0, sl]),
                "h1": htile(hidden_state[1, sl]),
            }
        )
    return in_maps


def _build_program(t_steps, unroll):
    import concourse.bass as bass
    import concourse.mybir as mybir
    import concourse.tile as tile
    from concourse import bacc
    from contextlib import ExitStack

    f32 = mybir.dt.float32
    bf16 = mybir.dt.bfloat16
    AF = mybir.ActivationFunctionType

    nc = bacc.Bacc("TRN2", target_bir_lowering=False, debug=False)

    p_whh0 = nc.declare_dram_parameter("whh0", [128, KT * MT * 128], bf16, isOutput=False)
    p_wih1 = nc.declare_dram_parameter("wih1", [128, KT * MT * 128], bf16, isOutput=False)
    p_whh1 = nc.declare_dram_parameter("whh1", [128, KT * MT * 128], bf16, isOutput=False)
    p_gxtab = nc.declare_dram_parameter("gxtab", [128, MT * 128], bf16, isOutput=False)
    p_embt = nc.declare_dram_parameter("embt", [128, KT * VOCAB], bf16, isOutput=False)
    p_bias = nc.declare_dram_parameter("bias", [1, BIAS_LEN], bf16, isOutput=False)
    p_ones = nc.declare_dram_parameter("ones", [1, BC], bf16, isOutput=False)
    p_oh = nc.declare_dram_parameter("oh", [VOCAB, T * BC], bf16, isOutput=False)
    p_h0 = nc.declare_dram_parameter("h0", [128, KT * BC], f32, isOutput=False)
    p_h1 = nc.declare_dram_parameter("h1", [128, KT * BC], f32, isOutput=False)
    p_out = nc.declare_dram_parameter("out", [t_steps * VOCAB, BC], f32, isOutput=True)

    with ExitStack() as ctx:
        tc = ctx.enter_context(tile.TileContext(nc))
        wpool = ctx.enter_context(tc.tile_pool(name="w", bufs=1))
        state = ctx.enter_context(tc.tile_pool(name="state", bufs=1))
        gates = ctx.enter_context(tc.tile_pool(name="gates", bufs=1))
        stage = ctx.enter_context(tc.tile_pool(name="stage", bufs=3))
        psum = ctx.enter_context(tc.tile_pool(name="psum", bufs=2, space="PSUM"))

        def load(pool, param, shape, dtype, tag):
            t_ = pool.tile(shape, dtype, tag=tag)
            nc.sync.dma_start(t_[:], param.ap())
            return t_

        s_whh0 = load(wpool, p_whh0, [128, KT * MT * 128], bf16, "whh0")
        s_wih1 = load(wpool, p_wih1, [128, KT * MT * 128], bf16, "wih1")
        s_whh1 = load(wpool, p_whh1, [128, KT * MT * 128], bf16, "whh1")
        s_gxtab = load(wpool, p_gxtab, [128, MT * 128], bf16, "gxtab")
        s_embt = load(wpool, p_embt, [128, KT * VOCAB], bf16, "embt")
        s_bias = load(wpool, p_bias, [1, BIAS_LEN], bf16, "bias")
        s_ones = load(wpool, p_ones, [1, BC], bf16, "ones")
        s_oh = load(wpool, p_oh, [VOCAB, T * BC], bf16, "oh_all")
        s_h0f = load(state, p_h0, [128, KT * BC], f32, "h0f")
        s_h1f = load(state, p_h1, [128, KT * BC], f32, "h1f")
        s_h0b = state.tile([128, KT * BC], bf16, tag="h0b")
        s_h1b = state.tile([128, KT * BC], bf16, tag="h1b")
        nc.vector.tensor_copy(s_h0b[:], s_h0f[:])
        nc.vector.tensor_copy(s_h1b[:], s_h1f[:])

        def wsl(s, k, m):  # weight lhsT tile (128, 128)
            o = (k * MT + m) * 128
            return s[:, o:o + 128]

        def hsl(s, k):  # h rhs tile (128, BC)
            return s[:, k * BC:(k + 1) * BC]

        def bank_slice(p, j):
            return p[:, j * BC:(j + 1) * BC]

        def gate_math(pr, pz, pnx, pnh, hf, hb):
            r_s = gates.tile([128, 8 * BC], f32, tag="rs")
            nc.scalar.activation(r_s[:], pr[:], AF.Sigmoid)
            z_s = gates.tile([128, 8 * BC], f32, tag="zs")
            nc.scalar.activation(z_s[:], pz[:], AF.Sigmoid)
            t1 = gates.tile([128, 8 * BC], f32, tag="tA")
            nc.vector.tensor_mul(t1[:], r_s[:], pnh[:])
            t2 = gates.tile([128, 8 * BC], f32, tag="tB")
            nc.vector.tensor_add(t2[:], pnx[:], t1[:])
            n_s = gates.tile([128, 8 * BC], f32, tag="ns")
            nc.scalar.activation(n_s[:], t2[:], AF.Tanh)
            d = gates.tile([128, 8 * BC], f32, tag="tA")
            nc.vector.tensor_sub(d[:], hf[:], n_s[:])
            m_ = gates.tile([128, 8 * BC], f32, tag="tB")
            nc.vector.tensor_mul(m_[:], z_s[:], d[:])
            nc.vector.tensor_add(hf[:], n_s[:], m_[:])
            nc.vector.tensor_copy(hb[:], hf[:])

        def bias_mm(pbank, j, boff, start):
            nc.tensor.matmul(
                bank_slice(pbank, j),
                s_bias[0:1, boff + j * 128: boff + (j + 1) * 128],
                s_ones[0:1, :],
                start=start,
                stop=False,
            )

        def step(t):
            oh_t = stage.tile([128, BC], bf16, tag="oh")
            nc.vector.tensor_copy(oh_t[:], s_oh[:, bass.ds(t * BC, BC)])

            # ---- layer 0 ----
            pr = psum.tile([128, 8 * BC], f32, tag="r")
            pz = psum.tile([128, 8 * BC], f32, tag="z")
            pnx = psum.tile([128, 8 * BC], f32, tag="nx")
            pnh = psum.tile([128, 8 * BC], f32, tag="nh")
            for j in range(8):  # r: table + gh chains
                nc.tensor.matmul(bank_slice(pr, j), s_gxtab[:, j * 128:(j + 1) * 128],
                                 oh_t[:], start=(j == 0), stop=False)
                for k in range(KT):
                    nc.tensor.matmul(bank_slice(pr, j), wsl(s_whh0, k, j),
                                     hsl(s_h0b, k), start=False,
                                     stop=(j == 7 and k == KT - 1))
            for j in range(8):  # z
                m = 8 + j
                nc.tensor.matmul(bank_slice(pz, j), s_gxtab[:, m * 128:(m + 1) * 128],
                                 oh_t[:], start=(j == 0), stop=False)
                for k in range(KT):
                    nc.tensor.matmul(bank_slice(pz, j), wsl(s_whh0, k, m),
                                     hsl(s_h0b, k), start=False,
                                     stop=(j == 7 and k == KT - 1))
            for j in range(8):  # nx: table only (b_ih0 baked in)
                m = 16 + j
                nc.tensor.matmul(bank_slice(pnx, j), s_gxtab[:, m * 128:(m + 1) * 128],
                                 oh_t[:], start=(j == 0), stop=(j == 7))
            for j in range(8):  # nh: bias + gh
                m = 16 + j
                bias_mm(pnh, j, OFF_NH0, start=(j == 0))
                for k in range(KT):
                    nc.tensor.matmul(bank_slice(pnh, j), wsl(s_whh0, k, m),
                                     hsl(s_h0b, k), start=False,
                                     stop=(j == 7 and k == KT - 1))
            gate_math(pr, pz, pnx, pnh, s_h0f, s_h0b)

            # ---- layer 1 ----
            pr1 = psum.tile([128, 8 * BC], f32, tag="r")
            pz1 = psum.tile([128, 8 * BC], f32, tag="z")
            pnx1 = psum.tile([128, 8 * BC], f32, tag="nx")
            pnh1 = psum.tile([128, 8 * BC], f32, tag="nh")
            for j in range(8):  # r = bias + gx(h0) + gh(h1)
                bias_mm(pr1, j, OFF_RZ1, start=(j == 0))
                for k in range(KT):
                    nc.tensor.matmul(bank_slice(pr1, j), wsl(s_wih1, k, j),
                                     hsl(s_h0b, k), start=False, stop=False)
                for k in range(KT):
                    nc.tensor.matmul(bank_slice(pr1, j), wsl(s_whh1, k, j),
                                     hsl(s_h1b, k), start=False,
                                     stop=(j == 7 and k == KT - 1))
            for j in range(8):  # z
                m = 8 + j
                bias_mm(pz1, j, OFF_RZ1 + 1024, start=(j == 0))
                for k in range(KT):
                    nc.tensor.matmul(bank_slice(pz1, j), wsl(s_wih1, k, m),
                                     hsl(s_h0b, k), start=False, stop=False)
                for k in range(KT):
                    nc.tensor.matmul(bank_slice(pz1, j), wsl(s_whh1, k, m),
                                     hsl(s_h1b, k), start=False,
                                     stop=(j == 7 and k == KT - 1))
            for j in range(8):  # nx
                m = 16 + j
                bias_mm(pnx1, j, OFF_NX1, start=(j == 0))
                for k in range(KT):
                    nc.tensor.matmul(bank_slice(pnx1, j), wsl(s_wih1, k, m),
                                     hsl(s_h0b, k), start=False,
                                     stop=(j == 7 and k == KT - 1))
            for j in range(8):  # nh
                m = 16 + j
                bias_mm(pnh1, j, OFF_NH1, start=(j == 0))
                for k in range(KT):
                    nc.tensor.matmul(bank_slice(pnh1, j), wsl(s_whh1, k, m),
                                     hsl(s_h1b, k), start=False,
                                     stop=(j == 7 and k == KT - 1))
            gate_math(pr1, pz1, pnx1, pnh1, s_h1f, s_h1b)

            # ---- logits ----
            plg = psum.tile([128, BC], f32, tag="nx")
            for k in range(KT):
                nc.tensor.matmul(plg[:], s_embt[:, k * VOCAB:(k + 1) * VOCAB],
                                 hsl(s_h1b, k), start=(k == 0), stop=(k == KT - 1))
            lsb = stage.tile([128, BC], f32, tag="lg")
            nc.scalar.copy(lsb[:], plg[:])
            nc.sync.dma_start(p_out.ap()[bass.ds(t * VOCAB, VOCAB), :], lsb[:])

        if unroll >= t_steps:
            for t in range(t_steps):
                step(t)
        else:
            import concourse.mybir as mybir_
            with tc.For_i(0, t_steps, unroll,
                          hint_engines=(mybir_.EngineType.PE,)) as tv:
                for u in range(unroll):
                    step(tv + u)

    nc.compile()
    return nc


_PROGRAM_CACHE = {}


def _get_program(t_steps, unroll):
    key = (t_steps, unroll)
    if key not in _PROGRAM_CACHE:
        _PROGRAM_CACHE[key] = _build_program(t_steps, unroll)
    return _PROGRAM_CACHE[key]


def run(inputs, t_steps=T, unroll=2, trace=False):
    """Run on 8 cores. Returns (full_output (B,T,V) f32, BassKernelResults)."""
    from concourse.bass_utils import run_bass_kernel_spmd

    in_maps = _prep_host(**inputs)
    if t_steps != T:
        for m in in_maps:
            pass  # oh stays full-size; kernel only reads first t_steps slices
    nc = _get_program(t_steps, unroll)
    res = run_bass_kernel_spmd(nc, in_maps, core_ids=list(range(NCORES)),
                               trace=trace)
    out = np.empty((B, t_steps, VOCAB), np.float32)
    for c in range(NCORES):
        arr = np.asarray(res.results[c]["out"]).reshape(t_steps, VOCAB, BC)
        out[c * BC:(c + 1) * BC] = arr.transpose(2, 0, 1)
    return out, res


def kernel(**inputs) -> np.ndarray:
    out, _ = run(inputs, t_steps=T, unroll=2, trace=False)
    return out


def bench(inputs, t_steps=T, unroll=2, iters=3):
    """Build the sharded jit once, run repeatedly, return (out, times_sec)."""
    import time

    import jax
    import jax.numpy as jnp
    from jax.sharding import Mesh, PartitionSpec
    from jax.experimental.shard_map import shard_map
    import concourse.mybir as mybir
    from concourse import bass2jax
    from concourse.bass2jax import _bass_exec_p, partition_id_tensor

    bass2jax.install_neuronx_cc_hook()
    in_maps = _prep_host(**inputs)
    nc = _get_program(t_steps, unroll)

    partition_name = nc.partition_id_tensor.name if nc.partition_id_tensor else None
    in_names, out_names, out_avals, zero_outs = [], [], [], []
    for alloc in nc.m.functions[0].allocations:
        if not isinstance(alloc, mybir.MemoryLocationSet):
            continue
        name = alloc.memorylocations[0].name
        if alloc.kind == "ExternalInput":
            if name != partition_name:
                in_names.append(name)
        elif alloc.kind == "ExternalOutput":
            out_names.append(name)
            shape = tuple(alloc.tensor_shape)
            dtype = mybir.dt.np(alloc.dtype)
            out_avals.append(jax.core.ShapedArray(shape, dtype))
            zero_outs.append(np.zeros(shape, dtype))
    n_params = len(in_names)
    n_outs = len(out_avals)
    all_in_names = list(in_names) + list(out_names)
    if partition_name is not None:
        all_in_names.append(partition_name)
    donate = tuple(range(n_params, n_params + n_outs))

    def _body(*args):
        operands = list(args)
        if partition_name is not None:
            operands.append(partition_id_tensor())
        outs = _bass_exec_p.bind(
            *operands,
            out_avals=tuple(out_avals),
            in_names=tuple(all_in_names),
            out_names=tuple(out_names),
            lowering_input_output_aliases=(),
            sim_require_finite=True,
            sim_require_nnan=True,
            nc=nc,
        )
        return tuple(outs)

    devices = jax.devices()[:NCORES]
    mesh = Mesh(np.asarray(devices), ("core",))
    in_specs = (PartitionSpec("core"),) * (n_params + n_outs)
    out_specs = (PartitionSpec("core"),) * len(out_names)
    sharded = jax.jit(
        shard_map(_body, mesh=mesh, in_specs=in_specs, out_specs=out_specs,
                  check_rep=False),
        donate_argnums=donate, keep_unused=True,
    )
    concat_in = [
        np.concatenate([np.asarray(in_maps[c][nm]) for c in range(NCORES)], axis=0)
        for nm in in_names
    ]
    sharding = jax.sharding.NamedSharding(mesh, PartitionSpec("core"))
    dev_in = [jax.device_put(a, sharding) for a in concat_in]

    def zeros():
        return [jax.device_put(
            np.zeros((NCORES * z.shape[0], *z.shape[1:]), z.dtype), sharding)
            for z in zero_outs]

    out_arrs = sharded(*dev_in, *zeros())
    jax.block_until_ready(out_arrs)
    times = []
    for _ in range(iters):
        zs = zeros()
        jax.block_until_ready(zs)
        t0 = time.perf_counter()
        out_arrs2 = sharded(*dev_in, *zs)
        jax.block_until_ready(out_arrs2)
        times.append(time.perf_counter() - t0)
        out_arrs = out_arrs2

    out = np.empty((B, t_steps, VOCAB), np.float32)
    full = np.asarray(out_arrs[0]).reshape(NCORES, t_steps, VOCAB, BC)
    for c in range(NCORES):
        out[c * BC:(c + 1) * BC] = full[c].transpose(2, 0, 1)
    return out, times


# revision 17
# speedup vs baseline: 38.3236x; 16.5931x over previous
"""Trainium2 Bass kernel for a 2-layer GRU phoneme decoder (teacher forcing).

Model: B=512, T=128, H=1024, L=2, V=128.
  tokens = [inp[:,0], target[:,1:]]          (B, T)
  per step t: x = emb[tokens[:,t]]; 2 GRU layers; logits = h1 @ emb.T

Strategy (per the data-parallel sharding hint):
  - Shard batch B=512 across 8 NeuronCores (64 rows each); weights replicated.
  - "Transposed world" layout: feature dims live on SBUF partitions, the
    64-row batch lives on the free axis. All matmuls are weight-stationary:
        out[gates_tile(128), batch(64)] += W_tile(128K,128M).T @ hT(128K,64)
  - Layer-0 input matmul is eliminated: gx0 = onehot(token) @ (emb @ W_ih0.T
    + b_ih0 + b_hh0[rz]) via a 128-row table (vocab on partitions, K=128).
  - Biases folded with K=1 rank-1 matmuls into the PSUM accumulation chains.
  - Gate math fp32 on full-width (128, 512) slabs; h state fp32, cast to bf16
    for the next matmul.
  - All weights stay resident in SBUF for the whole 128-step recurrence.
"""

import os
import sys

import numpy as np
import ml_dtypes

sys.path.insert(0, "/opt/trn_rl_repo")

VOCAB, H, L, B, T = 128, 1024, 2, 512, 128
NCORES = 8
BC = B // NCORES  # 64 batch rows per core
GH = 3 * H  # 3072 gates
MT = GH // 128  # 24 gate M-tiles
KT = H // 128  # 8 contraction K-tiles
BF = ml_dtypes.bfloat16

# bias vector layout (single row, bf16): [b_hh0_n | b_rz1 | b_ih1_n | b_hh1_n]
OFF_NH0 = 0
OFF_RZ1 = 1024
OFF_NX1 = 1024 + 2048
OFF_NH1 = 1024 + 2048 + 1024
BIAS_LEN = 1024 + 2048 + 1024 + 1024


def _prep_host(inp, target, hidden_state, embedding, W_ih, W_hh, b_ih, b_hh):
    """Pack full inputs into per-core in_maps (all hardware layouts)."""
    inp = np.asarray(inp)
    target = np.asarray(target)
    hidden_state = np.asarray(hidden_state, dtype=np.float32)
    embedding = np.asarray(embedding, dtype=np.float32)
    W_ih = np.asarray(W_ih, dtype=np.float32)
    W_hh = np.asarray(W_hh, dtype=np.float32)
    b_ih = np.asarray(b_ih, dtype=np.float32)
    b_hh = np.asarray(b_hh, dtype=np.float32)

    tokens = np.concatenate([inp[:, :1], target[:, 1:]], axis=1).astype(np.int64)

    def wtiles(W):  # (3072, 1024) -> (128, KT*MT*128) lhsT tiles [p][k,m,c]
        W4 = W.reshape(MT, 128, KT, 128)  # [m, c, k, p]
        return np.ascontiguousarray(
            W4.transpose(3, 2, 0, 1).reshape(128, KT * MT * 128)
        ).astype(BF)

    whh0 = wtiles(W_hh[0])
    wih1 = wtiles(W_ih[1])
    whh1 = wtiles(W_hh[1])

    # layer-0 input table: includes b_ih0 (all gates) and b_hh0 (r,z only)
    tab = embedding @ W_ih[0].T + b_ih[0]
    tab[:, : 2 * H] += b_hh[0][: 2 * H]
    gxtab = np.ascontiguousarray(tab).astype(BF)  # (128, 3072) [v][m*128+c]

    # logits lhsT tiles: [p][k*128+v] = emb[v, k*128+p]
    embt = np.ascontiguousarray(
        embedding.reshape(VOCAB, KT, 128).transpose(2, 1, 0).reshape(128, KT * VOCAB)
    ).astype(BF)

    bias = np.zeros((1, BIAS_LEN), np.float32)
    bias[0, OFF_NH0:OFF_NH0 + H] = b_hh[0][2 * H:]
    bias[0, OFF_RZ1:OFF_RZ1 + 2 * H] = b_ih[1][: 2 * H] + b_hh[1][: 2 * H]
    bias[0, OFF_NX1:OFF_NX1 + H] = b_ih[1][2 * H:]
    bias[0, OFF_NH1:OFF_NH1 + H] = b_hh[1][2 * H:]
    bias = bias.astype(BF)
    ones = np.ones((1, 4 * BC), BF)

    in_maps = []
    for c in range(NCORES):
        sl = slice(c * BC, (c + 1) * BC)
        tk = tokens[sl]  # (BC, T)
        oh = (np.arange(VOCAB)[:, None, None] == tk.T[None, :, :]).astype(BF)
        oh = np.ascontiguousarray(oh.reshape(VOCAB, T * BC))  # [v][t*BC+b]

        def htile(hl):  # (BC, 1024) -> (128, KT*BC) f32  [p][k*BC+b]
            return np.ascontiguousarray(
                hl.reshape(BC, KT, 128).transpose(2, 1, 0).reshape(128, KT * BC)
            ).astype(np.float32)

        in_maps.append(
            {
                "whh0": whh0,
                "wih1": wih1,
                "whh1": whh1,
                "gxtab": gxtab,
                "embt": embt,
                "bias": bias,
                "ones": ones,
                "oh": oh,
                "h0": htile(hidden_state[0, sl]),
                "h1": htile(hidden_state[1, sl]),
            }
        )
    return in_maps


def _build_program(t_steps, unroll, reps=1, ss=1, pipe=False):
    import concourse.bass as bass
    import concourse.mybir as mybir
    import concourse.tile as tile
    from concourse import bacc
    from contextlib import ExitStack

    f32 = mybir.dt.float32
    bf16 = mybir.dt.bfloat16
    AF = mybir.ActivationFunctionType

    nc = bacc.Bacc("TRN2", target_bir_lowering=False, debug=False)

    p_whh0 = nc.declare_dram_parameter("whh0", [128, KT * MT * 128], bf16, isOutput=False)
    p_wih1 = nc.declare_dram_parameter("wih1", [128, KT * MT * 128], bf16, isOutput=False)
    p_whh1 = nc.declare_dram_parameter("whh1", [128, KT * MT * 128], bf16, isOutput=False)
    p_gxtab = nc.declare_dram_parameter("gxtab", [128, MT * 128], bf16, isOutput=False)
    p_embt = nc.declare_dram_parameter("embt", [128, KT * VOCAB], bf16, isOutput=False)
    p_bias = nc.declare_dram_parameter("bias", [1, BIAS_LEN], bf16, isOutput=False)
    p_ones = nc.declare_dram_parameter("ones", [1, 4 * BC], bf16, isOutput=False)
    p_oh = nc.declare_dram_parameter("oh", [VOCAB, T * BC], bf16, isOutput=False)
    p_h0 = nc.declare_dram_parameter("h0", [128, KT * BC], f32, isOutput=False)
    p_h1 = nc.declare_dram_parameter("h1", [128, KT * BC], f32, isOutput=False)
    p_out = nc.declare_dram_parameter("out", [t_steps * VOCAB, BC], f32, isOutput=True)

    with ExitStack() as ctx:
        tc = ctx.enter_context(tile.TileContext(nc))
        wpool = ctx.enter_context(tc.tile_pool(name="w", bufs=1))
        state = ctx.enter_context(tc.tile_pool(name="state", bufs=1))
        gates = ctx.enter_context(tc.tile_pool(name="gates", bufs=1))
        stage = ctx.enter_context(tc.tile_pool(name="stage", bufs=3))
        psum = ctx.enter_context(tc.tile_pool(name="psum", bufs=2, space="PSUM"))

        def load(pool, param, shape, dtype, tag):
            t_ = pool.tile(shape, dtype, tag=tag)
            nc.sync.dma_start(t_[:], param.ap())
            return t_

        s_whh0 = load(wpool, p_whh0, [128, KT * MT * 128], bf16, "whh0")
        s_wih1 = load(wpool, p_wih1, [128, KT * MT * 128], bf16, "wih1")
        s_whh1 = load(wpool, p_whh1, [128, KT * MT * 128], bf16, "whh1")
        s_gxtab = load(wpool, p_gxtab, [128, MT * 128], bf16, "gxtab")
        s_embt = load(wpool, p_embt, [128, KT * VOCAB], bf16, "embt")
        s_bias = load(wpool, p_bias, [1, BIAS_LEN], bf16, "bias")
        s_ones = load(wpool, p_ones, [1, 4 * BC], bf16, "ones")
        s_oh = None
        if ss == 1:
            s_oh = load(wpool, p_oh, [VOCAB, T * BC], bf16, "oh_all")
        s_h0f = load(state, p_h0, [128, KT * BC], f32, "h0f")
        s_h1f = load(state, p_h1, [128, KT * BC], f32, "h1f")
        s_h0b = state.tile([128, KT * BC], bf16, tag="h0b")
        s_h1b = state.tile([128, KT * BC], bf16, tag="h1b")
        nc.vector.tensor_copy(s_h0b[:], s_h0f[:])
        nc.vector.tensor_copy(s_h1b[:], s_h1f[:])

        def wsl(s, k, m):  # weight lhsT tile (128, 128)
            o = (k * MT + m) * 128
            return s[:, o:o + 128]

        def hsl(s, k):  # h rhs tile (128, BC)
            return s[:, k * BC:(k + 1) * BC]

        def bank_slice(p, j):
            return p[:, j * BC:(j + 1) * BC]

        def gate_math(pr, pz, pnx, pnh, hf, hb):
            shp = list(pr.shape)
            r_s = gates.tile(shp, f32, tag="rs")
            nc.scalar.activation(r_s[:], pr[:], AF.Sigmoid)
            z_s = gates.tile(shp, f32, tag="zs")
            nc.scalar.activation(z_s[:], pz[:], AF.Sigmoid)
            t1 = gates.tile(shp, f32, tag="tA")
            nc.vector.tensor_mul(t1[:], r_s[:], pnh[:])
            t2 = gates.tile(shp, f32, tag="tB")
            nc.vector.tensor_add(t2[:], pnx[:], t1[:])
            n_s = gates.tile(shp, f32, tag="ns")
            nc.scalar.activation(n_s[:], t2[:], AF.Tanh)
            d = gates.tile(shp, f32, tag="tA")
            nc.vector.tensor_sub(d[:], hf[:], n_s[:])
            m_ = gates.tile(shp, f32, tag="tB")
            nc.vector.tensor_mul(m_[:], z_s[:], d[:])
            nc.vector.tensor_add(hf[:], n_s[:], m_[:])
            nc.vector.tensor_copy(hb[:], hf[:])

        def bias_mm(pbank, j, boff, start):
            nc.tensor.matmul(
                bank_slice(pbank, j),
                s_bias[0:1, boff + j * 128: boff + (j + 1) * 128],
                s_ones[0:1, 0:BC],
                start=start,
                stop=False,
            )

        def step(t):
            oh_t = stage.tile([128, BC], bf16, tag="oh")
            nc.vector.tensor_copy(oh_t[:], s_oh[:, bass.ds(t * BC, BC)])

            # ---- layer 0 ----
            pr = psum.tile([128, 8 * BC], f32, tag="r")
            pz = psum.tile([128, 8 * BC], f32, tag="z")
            pnx = psum.tile([128, 8 * BC], f32, tag="nx")
            pnh = psum.tile([128, 8 * BC], f32, tag="nh")
            for j in range(8):  # r: table + gh chains
                nc.tensor.matmul(bank_slice(pr, j), s_gxtab[:, j * 128:(j + 1) * 128],
                                 oh_t[:], start=(j == 0), stop=False)
                for k in range(KT):
                    nc.tensor.matmul(bank_slice(pr, j), wsl(s_whh0, k, j),
                                     hsl(s_h0b, k), start=False,
                                     stop=(j == 7 and k == KT - 1))
            for j in range(8):  # z
                m = 8 + j
                nc.tensor.matmul(bank_slice(pz, j), s_gxtab[:, m * 128:(m + 1) * 128],
                                 oh_t[:], start=(j == 0), stop=False)
                for k in range(KT):
                    nc.tensor.matmul(bank_slice(pz, j), wsl(s_whh0, k, m),
                                     hsl(s_h0b, k), start=False,
                                     stop=(j == 7 and k == KT - 1))
            for j in range(8):  # nx: table only (b_ih0 baked in)
                m = 16 + j
                nc.tensor.matmul(bank_slice(pnx, j), s_gxtab[:, m * 128:(m + 1) * 128],
                                 oh_t[:], start=(j == 0), stop=(j == 7))
            for j in range(8):  # nh: bias + gh
                m = 16 + j
                bias_mm(pnh, j, OFF_NH0, start=(j == 0))
                for k in range(KT):
                    nc.tensor.matmul(bank_slice(pnh, j), wsl(s_whh0, k, m),
                                     hsl(s_h0b, k), start=False,
                                     stop=(j == 7 and k == KT - 1))
            gate_math(pr, pz, pnx, pnh, s_h0f, s_h0b)

            # ---- layer 1 ----
            pr1 = psum.tile([128, 8 * BC], f32, tag="r")
            pz1 = psum.tile([128, 8 * BC], f32, tag="z")
            pnx1 = psum.tile([128, 8 * BC], f32, tag="nx")
            pnh1 = psum.tile([128, 8 * BC], f32, tag="nh")
            for j in range(8):  # r = bias + gx(h0) + gh(h1)
                bias_mm(pr1, j, OFF_RZ1, start=(j == 0))
                for k in range(KT):
                    nc.tensor.matmul(bank_slice(pr1, j), wsl(s_wih1, k, j),
                                     hsl(s_h0b, k), start=False, stop=False)
                for k in range(KT):
                    nc.tensor.matmul(bank_slice(pr1, j), wsl(s_whh1, k, j),
                                     hsl(s_h1b, k), start=False,
                                     stop=(j == 7 and k == KT - 1))
            for j in range(8):  # z
                m = 8 + j
                bias_mm(pz1, j, OFF_RZ1 + 1024, start=(j == 0))
                for k in range(KT):
                    nc.tensor.matmul(bank_slice(pz1, j), wsl(s_wih1, k, m),
                                     hsl(s_h0b, k), start=False, stop=False)
                for k in range(KT):
                    nc.tensor.matmul(bank_slice(pz1, j), wsl(s_whh1, k, m),
                                     hsl(s_h1b, k), start=False,
                                     stop=(j == 7 and k == KT - 1))
            for j in range(8):  # nx
                m = 16 + j
                bias_mm(pnx1, j, OFF_NX1, start=(j == 0))
                for k in range(KT):
                    nc.tensor.matmul(bank_slice(pnx1, j), wsl(s_wih1, k, m),
                                     hsl(s_h0b, k), start=False,
                                     stop=(j == 7 and k == KT - 1))
            for j in range(8):  # nh
                m = 16 + j
                bias_mm(pnh1, j, OFF_NH1, start=(j == 0))
                for k in range(KT):
                    nc.tensor.matmul(bank_slice(pnh1, j), wsl(s_whh1, k, m),
                                     hsl(s_h1b, k), start=False,
                                     stop=(j == 7 and k == KT - 1))
            gate_math(pr1, pz1, pnx1, pnh1, s_h1f, s_h1b)

            # ---- logits ----
            plg = psum.tile([128, BC], f32, tag="nx")
            for k in range(KT):
                nc.tensor.matmul(plg[:], s_embt[:, k * VOCAB:(k + 1) * VOCAB],
                                 hsl(s_h1b, k), start=(k == 0), stop=(k == KT - 1))
            lsb = stage.tile([128, BC], f32, tag="lg")
            nc.scalar.copy(lsb[:], plg[:])
            nc.sync.dma_start(p_out.ap()[bass.ds(t * VOCAB, VOCAB), :], lsb[:])

        # ---- superstep (ss >= 2): batch gx1 over ss steps, stage in f16 ----
        f16 = mybir.dt.float16
        SW = ss * BC  # batched moving width
        if ss > 1:
            s_pair = state.tile([128, KT, SW], bf16, tag="h0pair")
            s_gxs = state.tile([128, MT, SW], f16, tag="gxstage")
            CH = 512 // SW  # M-tiles per psum chunk
            NCH = MT // CH

        def l0_step(t, oh_u):
            pr = psum.tile([128, 8, BC], f32, tag="r")
            pz = psum.tile([128, 8, BC], f32, tag="z")
            pnx = psum.tile([128, 8, BC], f32, tag="nx", bufs=1)
            pnh = psum.tile([128, 8, BC], f32, tag="nh", bufs=1)
            for j in range(8):
                nc.tensor.matmul(pr[:, j, :], s_gxtab[:, j * 128:(j + 1) * 128],
                                 oh_u, start=(j == 0), stop=False)
                for k in range(KT):
                    nc.tensor.matmul(pr[:, j, :], wsl(s_whh0, k, j),
                                     hsl(s_h0b, k), start=False,
                                     stop=(j == 7 and k == KT - 1))
            for j in range(8):
                m = 8 + j
                nc.tensor.matmul(pz[:, j, :], s_gxtab[:, m * 128:(m + 1) * 128],
                                 oh_u, start=(j == 0), stop=False)
                for k in range(KT):
                    nc.tensor.matmul(pz[:, j, :], wsl(s_whh0, k, m),
                                     hsl(s_h0b, k), start=False,
                                     stop=(j == 7 and k == KT - 1))
            for j in range(8):
                m = 16 + j
                nc.tensor.matmul(pnx[:, j, :], s_gxtab[:, m * 128:(m + 1) * 128],
                                 oh_u, start=(j == 0), stop=(j == 7))
            for j in range(8):
                m = 16 + j
                nc.tensor.matmul(pnh[:, j, :],
                                 s_bias[0:1, OFF_NH0 + j * 128: OFF_NH0 + (j + 1) * 128],
                                 s_ones[0:1, 0:BC], start=(j == 0), stop=False)
                for k in range(KT):
                    nc.tensor.matmul(pnh[:, j, :], wsl(s_whh0, k, m),
                                     hsl(s_h0b, k), start=False,
                                     stop=(j == 7 and k == KT - 1))
            gate_math(pr, pz, pnx, pnh,
                      s_h0f.rearrange("p (k b) -> p k b", b=BC),
                      s_h0b.rearrange("p (k b) -> p k b", b=BC))

        def gate_math_stg(pr, pz, snx, pnh, hf, hb, sr, sz):
            ar = gates.tile([128, 8, BC], f32, tag="tA")
            nc.vector.tensor_add(ar[:], pr[:], sr)
            az = gates.tile([128, 8, BC], f32, tag="tB")
            nc.vector.tensor_add(az[:], pz[:], sz)
            r_s = gates.tile([128, 8, BC], f32, tag="rs")
            nc.scalar.activation(r_s[:], ar[:], AF.Sigmoid)
            z_s = gates.tile([128, 8, BC], f32, tag="zs")
            nc.scalar.activation(z_s[:], az[:], AF.Sigmoid)
            t1 = gates.tile([128, 8, BC], f32, tag="tA")
            nc.vector.tensor_mul(t1[:], r_s[:], pnh[:])
            t2 = gates.tile([128, 8, BC], f32, tag="tB")
            nc.vector.tensor_add(t2[:], t1[:], snx)
            n_s = gates.tile([128, 8, BC], f32, tag="ns")
            nc.scalar.activation(n_s[:], t2[:], AF.Tanh)
            d = gates.tile([128, 8, BC], f32, tag="tA")
            nc.vector.tensor_sub(d[:], hf[:], n_s[:])
            m_ = gates.tile([128, 8, BC], f32, tag="tB")
            nc.vector.tensor_mul(m_[:], z_s[:], d[:])
            nc.vector.tensor_add(hf[:], n_s[:], m_[:])
            nc.vector.tensor_copy(hb[:], hf[:])

        def l0_phase(tb):
            oh_ss = stage.tile([128, SW], bf16, tag="oh")
            nc.sync.dma_start(oh_ss[:], p_oh.ap()[:, bass.ds(tb * BC, SW)])
            hb3 = s_h0b.rearrange("p (k b) -> p k b", b=BC)
            for u in range(ss):
                l0_step(tb + u, oh_ss[:, u * BC:(u + 1) * BC])
                nc.vector.tensor_copy(s_pair[:, :, u * BC:(u + 1) * BC], hb3[:])

        def gx_phase():
            for c in range(NCH):
                pgx = psum.tile([128, CH, SW], f32, tag="gx")
                for mi in range(CH):
                    m = c * CH + mi
                    boff = OFF_RZ1 + m * 128 if m < 16 else OFF_NX1 + (m - 16) * 128
                    nc.tensor.matmul(pgx[:, mi, :],
                                     s_bias[0:1, boff:boff + 128],
                                     s_ones[0:1, 0:SW],
                                     start=(mi == 0), stop=False)
                    for k in range(KT):
                        nc.tensor.matmul(pgx[:, mi, :], wsl(s_wih1, k, m),
                                         s_pair[:, k, :], start=False,
                                         stop=(mi == CH - 1 and k == KT - 1))
                nc.scalar.copy(s_gxs[:, c * CH:(c + 1) * CH, :], pgx[:])

        def l1_phase(tb):
            for u in range(ss):
                t = tb + u
                usl = slice(u * BC, (u + 1) * BC)
                pr1 = psum.tile([128, 8, BC], f32, tag="r")
                pz1 = psum.tile([128, 8, BC], f32, tag="z")
                pnh1 = psum.tile([128, 8, BC], f32, tag="nh", bufs=1)
                for j in range(8):
                    for k in range(KT):
                        nc.tensor.matmul(pr1[:, j, :], wsl(s_whh1, k, j),
                                         hsl(s_h1b, k),
                                         start=(j == 0 and k == 0),
                                         stop=(j == 7 and k == KT - 1))
                for j in range(8):
                    m = 8 + j
                    for k in range(KT):
                        nc.tensor.matmul(pz1[:, j, :], wsl(s_whh1, k, m),
                                         hsl(s_h1b, k),
                                         start=(j == 0 and k == 0),
                                         stop=(j == 7 and k == KT - 1))
                for j in range(8):
                    m = 16 + j
                    nc.tensor.matmul(pnh1[:, j, :],
                                     s_bias[0:1, OFF_NH1 + j * 128: OFF_NH1 + (j + 1) * 128],
                                     s_ones[0:1, 0:BC], start=(j == 0), stop=False)
                    for k in range(KT):
                        nc.tensor.matmul(pnh1[:, j, :], wsl(s_whh1, k, m),
                                         hsl(s_h1b, k), start=False,
                                         stop=(j == 7 and k == KT - 1))
                gate_math_stg(pr1, pz1, s_gxs[:, 16:24, usl], pnh1,
                              s_h1f.rearrange("p (k b) -> p k b", b=BC),
                              s_h1b.rearrange("p (k b) -> p k b", b=BC),
                              s_gxs[:, 0:8, usl], s_gxs[:, 8:16, usl])
                plg = psum.tile([128, BC], f32, tag="nx", bufs=1)
                for k in range(KT):
                    nc.tensor.matmul(plg[:], s_embt[:, k * VOCAB:(k + 1) * VOCAB],
                                     hsl(s_h1b, k), start=(k == 0),
                                     stop=(k == KT - 1))
                lsb = stage.tile([128, BC], f32, tag="lg")
                nc.scalar.copy(lsb[:], plg[:])
                nc.sync.dma_start(p_out.ap()[bass.ds(t * VOCAB, VOCAB), :], lsb[:])

        def whole_pipelined():
            # prologue: L0 + gx for the first superstep
            l0_phase(0)
            gx_phase()
            n_loop = t_steps // ss - 1
            if n_loop > 0:
                if unroll >= n_loop:
                    for i in range(n_loop):
                        tb = i * ss
                        l0_phase(tb + ss)
                        l1_phase(tb)
                        gx_phase()
                else:
                    with tc.For_i(0, n_loop * ss, ss,
                                  hint_engines=(mybir.EngineType.PE,)) as tv:
                        l0_phase(tv + ss)
                        l1_phase(tv)
                        gx_phase()
            l1_phase(t_steps - ss)

        def whole_seq():
            n_super = t_steps // ss
            if unroll >= n_super:
                for i in range(n_super):
                    tb = i * ss
                    l0_phase(tb)
                    gx_phase()
                    l1_phase(tb)
            else:
                with tc.For_i(0, t_steps, ss * unroll,
                              hint_engines=(mybir.EngineType.PE,)) as tv:
                    for v in range(unroll):
                        l0_phase(tv + v * ss)
                        gx_phase()
                        l1_phase(tv + v * ss)

        whole = whole_pipelined if pipe else whole_seq
        if ss > 1:
            if reps == 1:
                whole()
            else:
                with tc.For_i(0, reps, 1) as _rep:
                    whole()
        elif unroll >= t_steps:
            for t in range(t_steps):
                step(t)
        elif reps == 1:
            with tc.For_i(0, t_steps, unroll,
                          hint_engines=(mybir.EngineType.PE,)) as tv:
                for u in range(unroll):
                    step(tv + u)
        else:
            with tc.For_i(0, reps, 1) as _rep:
                with tc.For_i(0, t_steps, unroll,
                              hint_engines=(mybir.EngineType.PE,)) as tv:
                    for u in range(unroll):
                        step(tv + u)

    nc.compile()
    return nc


_PROGRAM_CACHE = {}


def _get_program(t_steps, unroll, reps=1, ss=1, pipe=False):
    key = (t_steps, unroll, reps, ss, pipe)
    if key not in _PROGRAM_CACHE:
        _PROGRAM_CACHE[key] = _build_program(t_steps, unroll, reps, ss, pipe)
    return _PROGRAM_CACHE[key]


def run(inputs, t_steps=T, unroll=2, trace=False, ss=1, pipe=False):
    """Run on 8 cores. Returns (full_output (B,T,V) f32, BassKernelResults)."""
    from concourse.bass_utils import run_bass_kernel_spmd

    in_maps = _prep_host(**inputs)
    if t_steps != T:
        for m in in_maps:
            pass  # oh stays full-size; kernel only reads first t_steps slices
    nc = _get_program(t_steps, unroll, 1, ss, pipe)
    res = run_bass_kernel_spmd(nc, in_maps, core_ids=list(range(NCORES)),
                               trace=trace)
    out = np.empty((B, t_steps, VOCAB), np.float32)
    for c in range(NCORES):
        arr = np.asarray(res.results[c]["out"]).reshape(t_steps, VOCAB, BC)
        out[c * BC:(c + 1) * BC] = arr.transpose(2, 0, 1)
    return out, res


def kernel(**inputs) -> np.ndarray:
    out, _ = run(inputs, t_steps=T, unroll=2, trace=False, ss=2)
    return out


def bench(inputs, t_steps=T, unroll=2, iters=3, reps=1, ss=1, pipe=False):
    """Build the sharded jit once, run repeatedly, return (out, times_sec)."""
    import time

    import jax
    import jax.numpy as jnp
    from jax.sharding import Mesh, PartitionSpec
    from jax.experimental.shard_map import shard_map
    import concourse.mybir as mybir
    from concourse import bass2jax
    from concourse.bass2jax import _bass_exec_p, partition_id_tensor

    bass2jax.install_neuronx_cc_hook()
    in_maps = _prep_host(**inputs)
    nc = _get_program(t_steps, unroll, reps, ss, pipe)

    partition_name = nc.partition_id_tensor.name if nc.partition_id_tensor else None
    in_names, out_names, out_avals, zero_outs = [], [], [], []
    for alloc in nc.m.functions[0].allocations:
        if not isinstance(alloc, mybir.MemoryLocationSet):
            continue
        name = alloc.memorylocations[0].name
        if alloc.kind == "ExternalInput":
            if name != partition_name:
                in_names.append(name)
        elif alloc.kind == "ExternalOutput":
            out_names.append(name)
            shape = tuple(alloc.tensor_shape)
            dtype = mybir.dt.np(alloc.dtype)
            out_avals.append(jax.core.ShapedArray(shape, dtype))
            zero_outs.append(np.zeros(shape, dtype))
    n_params = len(in_names)
    n_outs = len(out_avals)
    all_in_names = list(in_names) + list(out_names)
    if partition_name is not None:
        all_in_names.append(partition_name)
    donate = tuple(range(n_params, n_params + n_outs))

    def _body(*args):
        operands = list(args)
        if partition_name is not None:
            operands.append(partition_id_tensor())
        outs = _bass_exec_p.bind(
            *operands,
            out_avals=tuple(out_avals),
            in_names=tuple(all_in_names),
            out_names=tuple(out_names),
            lowering_input_output_aliases=(),
            sim_require_finite=True,
            sim_require_nnan=True,
            nc=nc,
        )
        return tuple(outs)

    devices = jax.devices()[:NCORES]
    mesh = Mesh(np.asarray(devices), ("core",))
    in_specs = (PartitionSpec("core"),) * (n_params + n_outs)
    out_specs = (PartitionSpec("core"),) * len(out_names)
    sharded = jax.jit(
        shard_map(_body, mesh=mesh, in_specs=in_specs, out_specs=out_specs,
                  check_rep=False),
        donate_argnums=donate, keep_unused=True,
    )
    concat_in = [
        np.concatenate([np.asarray(in_maps[c][nm]) for c in range(NCORES)], axis=0)
        for nm in in_names
    ]
    sharding = jax.sharding.NamedSharding(mesh, PartitionSpec("core"))
    dev_in = [jax.device_put(a, sharding) for a in concat_in]

    def zeros():
        return [jax.device_put(
            np.zeros((NCORES * z.shape[0], *z.shape[1:]), z.dtype), sharding)
            for z in zero_outs]

    out_arrs = sharded(*dev_in, *zeros())
    jax.block_until_ready(out_arrs)
    times = []
    for _ in range(iters):
        zs = zeros()
        jax.block_until_ready(zs)
        t0 = time.perf_counter()
        out_arrs2 = sharded(*dev_in, *zs)
        jax.block_until_ready(out_arrs2)
        times.append(time.perf_counter() - t0)
        out_arrs = out_arrs2

    out = np.empty((B, t_steps, VOCAB), np.float32)
    full = np.asarray(out_arrs[0]).reshape(NCORES, t_steps, VOCAB, BC)
    for c in range(NCORES):
        out[c * BC:(c + 1) * BC] = full[c].transpose(2, 0, 1)
    return out, times


# revision 18
# speedup vs baseline: 42.0590x; 1.0975x over previous
"""Trainium2 Bass kernel for a 2-layer GRU phoneme decoder (teacher forcing).

Model: B=512, T=128, H=1024, L=2, V=128.
  tokens = [inp[:,0], target[:,1:]]          (B, T)
  per step t: x = emb[tokens[:,t]]; 2 GRU layers; logits = h1 @ emb.T

Strategy (per the data-parallel sharding hint):
  - Shard batch B=512 across 8 NeuronCores (64 rows each); weights replicated.
  - "Transposed world" layout: feature dims live on SBUF partitions, the
    64-row batch lives on the free axis. All matmuls are weight-stationary:
        out[gates_tile(128), batch(64)] += W_tile(128K,128M).T @ hT(128K,64)
  - Layer-0 input matmul is eliminated: gx0 = onehot(token) @ (emb @ W_ih0.T
    + b_ih0 + b_hh0[rz]) via a 128-row table (vocab on partitions, K=128).
  - Biases folded with K=1 rank-1 matmuls into the PSUM accumulation chains.
  - Gate math fp32 on full-width (128, 512) slabs; h state fp32, cast to bf16
    for the next matmul.
  - All weights stay resident in SBUF for the whole 128-step recurrence.
  - Superstep mode (ss>=2): the layer-1 input matmul gx1 = W_ih1 @ h0 is
    batched over ss consecutive steps (moving width ss*64), staged to SBUF in
    f16, and added to the gh1 PSUM banks during the layer-1 gate math. This
    cuts LDWEIGHTS traffic, the dominant cost: every weight tile must be
    re-streamed into the PE array each step and gets only 64 reuse columns,
    so the kernel sits at the weight-load bandwidth wall (~2 cols/cycle).

Measured (differential wall-clock over 20-30 in-NEFF repetitions, 8 cores):
  v1 (ss=1): ~4.57 ms; ss=2: ~4.26-4.45 ms. FLOP roofline ~2.0 ms;
  weight-reload wall for this layout ~3.6 ms.
"""

import os
import sys

import numpy as np
import ml_dtypes

sys.path.insert(0, "/opt/trn_rl_repo")

VOCAB, H, L, B, T = 128, 1024, 2, 512, 128
NCORES = 8
BC = B // NCORES  # 64 batch rows per core
GH = 3 * H  # 3072 gates
MT = GH // 128  # 24 gate M-tiles
KT = H // 128  # 8 contraction K-tiles
BF = ml_dtypes.bfloat16

# bias vector layout (single row, bf16): [b_hh0_n | b_rz1 | b_ih1_n | b_hh1_n]
OFF_NH0 = 0
OFF_RZ1 = 1024
OFF_NX1 = 1024 + 2048
OFF_NH1 = 1024 + 2048 + 1024
BIAS_LEN = 1024 + 2048 + 1024 + 1024


def _prep_host(inp, target, hidden_state, embedding, W_ih, W_hh, b_ih, b_hh):
    """Pack full inputs into per-core in_maps (all hardware layouts)."""
    inp = np.asarray(inp)
    target = np.asarray(target)
    hidden_state = np.asarray(hidden_state, dtype=np.float32)
    embedding = np.asarray(embedding, dtype=np.float32)
    W_ih = np.asarray(W_ih, dtype=np.float32)
    W_hh = np.asarray(W_hh, dtype=np.float32)
    b_ih = np.asarray(b_ih, dtype=np.float32)
    b_hh = np.asarray(b_hh, dtype=np.float32)

    tokens = np.concatenate([inp[:, :1], target[:, 1:]], axis=1).astype(np.int64)

    def wtiles(W):  # (3072, 1024) -> (128, KT*MT*128) lhsT tiles [p][k,m,c]
        W4 = W.reshape(MT, 128, KT, 128)  # [m, c, k, p]
        return np.ascontiguousarray(
            W4.transpose(3, 2, 0, 1).reshape(128, KT * MT * 128)
        ).astype(BF)

    whh0 = wtiles(W_hh[0])
    wih1 = wtiles(W_ih[1])
    whh1 = wtiles(W_hh[1])

    # layer-0 input table: includes b_ih0 (all gates) and b_hh0 (r,z only)
    tab = embedding @ W_ih[0].T + b_ih[0]
    tab[:, : 2 * H] += b_hh[0][: 2 * H]
    gxtab = np.ascontiguousarray(tab).astype(BF)  # (128, 3072) [v][m*128+c]

    # logits lhsT tiles: [p][k*128+v] = emb[v, k*128+p]
    embt = np.ascontiguousarray(
        embedding.reshape(VOCAB, KT, 128).transpose(2, 1, 0).reshape(128, KT * VOCAB)
    ).astype(BF)

    bias = np.zeros((1, BIAS_LEN), np.float32)
    bias[0, OFF_NH0:OFF_NH0 + H] = b_hh[0][2 * H:]
    bias[0, OFF_RZ1:OFF_RZ1 + 2 * H] = b_ih[1][: 2 * H] + b_hh[1][: 2 * H]
    bias[0, OFF_NX1:OFF_NX1 + H] = b_ih[1][2 * H:]
    bias[0, OFF_NH1:OFF_NH1 + H] = b_hh[1][2 * H:]
    bias = bias.astype(BF)
    ones = np.ones((1, 4 * BC), BF)

    in_maps = []
    for c in range(NCORES):
        sl = slice(c * BC, (c + 1) * BC)
        tk = tokens[sl]  # (BC, T)
        oh = (np.arange(VOCAB)[:, None, None] == tk.T[None, :, :]).astype(BF)
        oh = np.ascontiguousarray(oh.reshape(VOCAB, T * BC))  # [v][t*BC+b]

        def htile(hl):  # (BC, 1024) -> (128, KT*BC) f32  [p][k*BC+b]
            return np.ascontiguousarray(
                hl.reshape(BC, KT, 128).transpose(2, 1, 0).reshape(128, KT * BC)
            ).astype(np.float32)

        in_maps.append(
            {
                "whh0": whh0,
                "wih1": wih1,
                "whh1": whh1,
                "gxtab": gxtab,
                "embt": embt,
                "bias": bias,
                "ones": ones,
                "oh": oh,
                "h0": htile(hidden_state[0, sl]),
                "h1": htile(hidden_state[1, sl]),
            }
        )
    return in_maps


def _build_program(t_steps, unroll, reps=1, ss=1, pipe=False):
    import concourse.bass as bass
    import concourse.mybir as mybir
    import concourse.tile as tile
    from concourse import bacc
    from contextlib import ExitStack

    f32 = mybir.dt.float32
    bf16 = mybir.dt.bfloat16
    AF = mybir.ActivationFunctionType

    nc = bacc.Bacc("TRN2", target_bir_lowering=False, debug=False)

    p_whh0 = nc.declare_dram_parameter("whh0", [128, KT * MT * 128], bf16, isOutput=False)
    p_wih1 = nc.declare_dram_parameter("wih1", [128, KT * MT * 128], bf16, isOutput=False)
    p_whh1 = nc.declare_dram_parameter("whh1", [128, KT * MT * 128], bf16, isOutput=False)
    p_gxtab = nc.declare_dram_parameter("gxtab", [128, MT * 128], bf16, isOutput=False)
    p_embt = nc.declare_dram_parameter("embt", [128, KT * VOCAB], bf16, isOutput=False)
    p_bias = nc.declare_dram_parameter("bias", [1, BIAS_LEN], bf16, isOutput=False)
    p_ones = nc.declare_dram_parameter("ones", [1, 4 * BC], bf16, isOutput=False)
    p_oh = nc.declare_dram_parameter("oh", [VOCAB, T * BC], bf16, isOutput=False)
    p_h0 = nc.declare_dram_parameter("h0", [128, KT * BC], f32, isOutput=False)
    p_h1 = nc.declare_dram_parameter("h1", [128, KT * BC], f32, isOutput=False)
    p_out = nc.declare_dram_parameter("out", [t_steps * VOCAB, BC], f32, isOutput=True)

    with ExitStack() as ctx:
        tc = ctx.enter_context(tile.TileContext(nc))
        wpool = ctx.enter_context(tc.tile_pool(name="w", bufs=1))
        state = ctx.enter_context(tc.tile_pool(name="state", bufs=1))
        gates = ctx.enter_context(tc.tile_pool(name="gates", bufs=1))
        stage = ctx.enter_context(tc.tile_pool(name="stage", bufs=3))
        psum = ctx.enter_context(tc.tile_pool(name="psum", bufs=2, space="PSUM"))

        def load(pool, param, shape, dtype, tag):
            t_ = pool.tile(shape, dtype, tag=tag)
            nc.sync.dma_start(t_[:], param.ap())
            return t_

        s_whh0 = load(wpool, p_whh0, [128, KT * MT * 128], bf16, "whh0")
        s_wih1 = load(wpool, p_wih1, [128, KT * MT * 128], bf16, "wih1")
        s_whh1 = load(wpool, p_whh1, [128, KT * MT * 128], bf16, "whh1")
        s_gxtab = load(wpool, p_gxtab, [128, MT * 128], bf16, "gxtab")
        s_embt = load(wpool, p_embt, [128, KT * VOCAB], bf16, "embt")
        s_bias = load(wpool, p_bias, [1, BIAS_LEN], bf16, "bias")
        s_ones = load(wpool, p_ones, [1, 4 * BC], bf16, "ones")
        s_oh = None
        if ss == 1:
            s_oh = load(wpool, p_oh, [VOCAB, T * BC], bf16, "oh_all")
        s_h0f = load(state, p_h0, [128, KT * BC], f32, "h0f")
        s_h1f = load(state, p_h1, [128, KT * BC], f32, "h1f")
        s_h0b = state.tile([128, KT * BC], bf16, tag="h0b")
        s_h1b = state.tile([128, KT * BC], bf16, tag="h1b")
        nc.vector.tensor_copy(s_h0b[:], s_h0f[:])
        nc.vector.tensor_copy(s_h1b[:], s_h1f[:])

        def wsl(s, k, m):  # weight lhsT tile (128, 128)
            o = (k * MT + m) * 128
            return s[:, o:o + 128]

        def hsl(s, k):  # h rhs tile (128, BC)
            return s[:, k * BC:(k + 1) * BC]

        def bank_slice(p, j):
            return p[:, j * BC:(j + 1) * BC]

        def gate_math(pr, pz, pnx, pnh, hf, hb):
            shp = list(pr.shape)
            r_s = gates.tile(shp, f32, tag="rs")
            nc.scalar.activation(r_s[:], pr[:], AF.Sigmoid)
            z_s = gates.tile(shp, f32, tag="zs")
            nc.scalar.activation(z_s[:], pz[:], AF.Sigmoid)
            t1 = gates.tile(shp, f32, tag="tA")
            nc.vector.tensor_mul(t1[:], r_s[:], pnh[:])
            t2 = gates.tile(shp, f32, tag="tB")
            nc.vector.tensor_add(t2[:], pnx[:], t1[:])
            n_s = gates.tile(shp, f32, tag="ns")
            nc.scalar.activation(n_s[:], t2[:], AF.Tanh)
            d = gates.tile(shp, f32, tag="tA")
            nc.vector.tensor_sub(d[:], hf[:], n_s[:])
            m_ = gates.tile(shp, f32, tag="tB")
            nc.vector.tensor_mul(m_[:], z_s[:], d[:])
            nc.vector.tensor_add(hf[:], n_s[:], m_[:])
            nc.vector.tensor_copy(hb[:], hf[:])

        def bias_mm(pbank, j, boff, start):
            nc.tensor.matmul(
                bank_slice(pbank, j),
                s_bias[0:1, boff + j * 128: boff + (j + 1) * 128],
                s_ones[0:1, 0:BC],
                start=start,
                stop=False,
            )

        def step(t):
            oh_t = stage.tile([128, BC], bf16, tag="oh")
            nc.vector.tensor_copy(oh_t[:], s_oh[:, bass.ds(t * BC, BC)])

            # ---- layer 0 ----
            pr = psum.tile([128, 8 * BC], f32, tag="r")
            pz = psum.tile([128, 8 * BC], f32, tag="z")
            pnx = psum.tile([128, 8 * BC], f32, tag="nx")
            pnh = psum.tile([128, 8 * BC], f32, tag="nh")
            for j in range(8):  # r: table + gh chains
                nc.tensor.matmul(bank_slice(pr, j), s_gxtab[:, j * 128:(j + 1) * 128],
                                 oh_t[:], start=(j == 0), stop=False)
                for k in range(KT):
                    nc.tensor.matmul(bank_slice(pr, j), wsl(s_whh0, k, j),
                                     hsl(s_h0b, k), start=False,
                                     stop=(j == 7 and k == KT - 1))
            for j in range(8):  # z
                m = 8 + j
                nc.tensor.matmul(bank_slice(pz, j), s_gxtab[:, m * 128:(m + 1) * 128],
                                 oh_t[:], start=(j == 0), stop=False)
                for k in range(KT):
                    nc.tensor.matmul(bank_slice(pz, j), wsl(s_whh0, k, m),
                                     hsl(s_h0b, k), start=False,
                                     stop=(j == 7 and k == KT - 1))
            for j in range(8):  # nx: table only (b_ih0 baked in)
                m = 16 + j
                nc.tensor.matmul(bank_slice(pnx, j), s_gxtab[:, m * 128:(m + 1) * 128],
                                 oh_t[:], start=(j == 0), stop=(j == 7))
            for j in range(8):  # nh: bias + gh
                m = 16 + j
                bias_mm(pnh, j, OFF_NH0, start=(j == 0))
                for k in range(KT):
                    nc.tensor.matmul(bank_slice(pnh, j), wsl(s_whh0, k, m),
                                     hsl(s_h0b, k), start=False,
                                     stop=(j == 7 and k == KT - 1))
            gate_math(pr, pz, pnx, pnh, s_h0f, s_h0b)

            # ---- layer 1 ----
            pr1 = psum.tile([128, 8 * BC], f32, tag="r")
            pz1 = psum.tile([128, 8 * BC], f32, tag="z")
            pnx1 = psum.tile([128, 8 * BC], f32, tag="nx")
            pnh1 = psum.tile([128, 8 * BC], f32, tag="nh")
            for j in range(8):  # r = bias + gx(h0) + gh(h1)
                bias_mm(pr1, j, OFF_RZ1, start=(j == 0))
                for k in range(KT):
                    nc.tensor.matmul(bank_slice(pr1, j), wsl(s_wih1, k, j),
                                     hsl(s_h0b, k), start=False, stop=False)
                for k in range(KT):
                    nc.tensor.matmul(bank_slice(pr1, j), wsl(s_whh1, k, j),
                                     hsl(s_h1b, k), start=False,
                                     stop=(j == 7 and k == KT - 1))
            for j in range(8):  # z
                m = 8 + j
                bias_mm(pz1, j, OFF_RZ1 + 1024, start=(j == 0))
                for k in range(KT):
                    nc.tensor.matmul(bank_slice(pz1, j), wsl(s_wih1, k, m),
                                     hsl(s_h0b, k), start=False, stop=False)
                for k in range(KT):
                    nc.tensor.matmul(bank_slice(pz1, j), wsl(s_whh1, k, m),
                                     hsl(s_h1b, k), start=False,
                                     stop=(j == 7 and k == KT - 1))
            for j in range(8):  # nx
                m = 16 + j
                bias_mm(pnx1, j, OFF_NX1, start=(j == 0))
                for k in range(KT):
                    nc.tensor.matmul(bank_slice(pnx1, j), wsl(s_wih1, k, m),
                                     hsl(s_h0b, k), start=False,
                                     stop=(j == 7 and k == KT - 1))
            for j in range(8):  # nh
                m = 16 + j
                bias_mm(pnh1, j, OFF_NH1, start=(j == 0))
                for k in range(KT):
                    nc.tensor.matmul(bank_slice(pnh1, j), wsl(s_whh1, k, m),
                                     hsl(s_h1b, k), start=False,
                                     stop=(j == 7 and k == KT - 1))
            gate_math(pr1, pz1, pnx1, pnh1, s_h1f, s_h1b)

            # ---- logits ----
            plg = psum.tile([128, BC], f32, tag="nx")
            for k in range(KT):
                nc.tensor.matmul(plg[:], s_embt[:, k * VOCAB:(k + 1) * VOCAB],
                                 hsl(s_h1b, k), start=(k == 0), stop=(k == KT - 1))
            lsb = stage.tile([128, BC], f32, tag="lg")
            nc.scalar.copy(lsb[:], plg[:])
            nc.sync.dma_start(p_out.ap()[bass.ds(t * VOCAB, VOCAB), :], lsb[:])

        # ---- superstep (ss >= 2): batch gx1 over ss steps, stage in f16 ----
        f16 = mybir.dt.float16
        SW = ss * BC  # batched moving width
        if ss > 1:
            s_pair = state.tile([128, KT, SW], bf16, tag="h0pair")
            s_gxs = state.tile([128, MT, SW], f16, tag="gxstage")
            CH = 512 // SW  # M-tiles per psum chunk
            NCH = MT // CH

        def l0_step(t, oh_u):
            pr = psum.tile([128, 8, BC], f32, tag="r")
            pz = psum.tile([128, 8, BC], f32, tag="z")
            pnx = psum.tile([128, 8, BC], f32, tag="nx", bufs=1)
            pnh = psum.tile([128, 8, BC], f32, tag="nh", bufs=1)
            for j in range(8):
                nc.tensor.matmul(pr[:, j, :], s_gxtab[:, j * 128:(j + 1) * 128],
                                 oh_u, start=(j == 0), stop=False)
                for k in range(KT):
                    nc.tensor.matmul(pr[:, j, :], wsl(s_whh0, k, j),
                                     hsl(s_h0b, k), start=False,
                                     stop=(j == 7 and k == KT - 1))
            for j in range(8):
                m = 8 + j
                nc.tensor.matmul(pz[:, j, :], s_gxtab[:, m * 128:(m + 1) * 128],
                                 oh_u, start=(j == 0), stop=False)
                for k in range(KT):
                    nc.tensor.matmul(pz[:, j, :], wsl(s_whh0, k, m),
                                     hsl(s_h0b, k), start=False,
                                     stop=(j == 7 and k == KT - 1))
            for j in range(8):
                m = 16 + j
                nc.tensor.matmul(pnx[:, j, :], s_gxtab[:, m * 128:(m + 1) * 128],
                                 oh_u, start=(j == 0), stop=(j == 7))
            for j in range(8):
                m = 16 + j
                nc.tensor.matmul(pnh[:, j, :],
                                 s_bias[0:1, OFF_NH0 + j * 128: OFF_NH0 + (j + 1) * 128],
                                 s_ones[0:1, 0:BC], start=(j == 0), stop=False)
                for k in range(KT):
                    nc.tensor.matmul(pnh[:, j, :], wsl(s_whh0, k, m),
                                     hsl(s_h0b, k), start=False,
                                     stop=(j == 7 and k == KT - 1))
            gate_math(pr, pz, pnx, pnh,
                      s_h0f.rearrange("p (k b) -> p k b", b=BC),
                      s_h0b.rearrange("p (k b) -> p k b", b=BC))

        def gate_math_stg(pr, pz, snx, pnh, hf, hb, sr, sz):
            ar = gates.tile([128, 8, BC], f32, tag="tA")
            nc.vector.tensor_add(ar[:], pr[:], sr)
            az = gates.tile([128, 8, BC], f32, tag="tB")
            nc.vector.tensor_add(az[:], pz[:], sz)
            r_s = gates.tile([128, 8, BC], f32, tag="rs")
            nc.scalar.activation(r_s[:], ar[:], AF.Sigmoid)
            z_s = gates.tile([128, 8, BC], f32, tag="zs")
            nc.scalar.activation(z_s[:], az[:], AF.Sigmoid)
            t1 = gates.tile([128, 8, BC], f32, tag="tA")
            nc.vector.tensor_mul(t1[:], r_s[:], pnh[:])
            t2 = gates.tile([128, 8, BC], f32, tag="tB")
            nc.vector.tensor_add(t2[:], t1[:], snx)
            n_s = gates.tile([128, 8, BC], f32, tag="ns")
            nc.scalar.activation(n_s[:], t2[:], AF.Tanh)
            d = gates.tile([128, 8, BC], f32, tag="tA")
            nc.vector.tensor_sub(d[:], hf[:], n_s[:])
            m_ = gates.tile([128, 8, BC], f32, tag="tB")
            nc.vector.tensor_mul(m_[:], z_s[:], d[:])
            nc.vector.tensor_add(hf[:], n_s[:], m_[:])
            nc.vector.tensor_copy(hb[:], hf[:])

        def l0_phase(tb):
            oh_ss = stage.tile([128, SW], bf16, tag="oh")
            nc.sync.dma_start(oh_ss[:], p_oh.ap()[:, bass.ds(tb * BC, SW)])
            hb3 = s_h0b.rearrange("p (k b) -> p k b", b=BC)
            for u in range(ss):
                l0_step(tb + u, oh_ss[:, u * BC:(u + 1) * BC])
                nc.vector.tensor_copy(s_pair[:, :, u * BC:(u + 1) * BC], hb3[:])

        def gx_phase():
            for c in range(NCH):
                pgx = psum.tile([128, CH, SW], f32, tag="gx")
                for mi in range(CH):
                    m = c * CH + mi
                    boff = OFF_RZ1 + m * 128 if m < 16 else OFF_NX1 + (m - 16) * 128
                    nc.tensor.matmul(pgx[:, mi, :],
                                     s_bias[0:1, boff:boff + 128],
                                     s_ones[0:1, 0:SW],
                                     start=(mi == 0), stop=False)
                    for k in range(KT):
                        nc.tensor.matmul(pgx[:, mi, :], wsl(s_wih1, k, m),
                                         s_pair[:, k, :], start=False,
                                         stop=(mi == CH - 1 and k == KT - 1))
                nc.scalar.copy(s_gxs[:, c * CH:(c + 1) * CH, :], pgx[:])

        def l1_phase(tb):
            for u in range(ss):
                t = tb + u
                usl = slice(u * BC, (u + 1) * BC)
                pr1 = psum.tile([128, 8, BC], f32, tag="r")
                pz1 = psum.tile([128, 8, BC], f32, tag="z")
                pnh1 = psum.tile([128, 8, BC], f32, tag="nh", bufs=1)
                for j in range(8):
                    for k in range(KT):
                        nc.tensor.matmul(pr1[:, j, :], wsl(s_whh1, k, j),
                                         hsl(s_h1b, k),
                                         start=(j == 0 and k == 0),
                                         stop=(j == 7 and k == KT - 1))
                for j in range(8):
                    m = 8 + j
                    for k in range(KT):
                        nc.tensor.matmul(pz1[:, j, :], wsl(s_whh1, k, m),
                                         hsl(s_h1b, k),
                                         start=(j == 0 and k == 0),
                                         stop=(j == 7 and k == KT - 1))
                for j in range(8):
                    m = 16 + j
                    nc.tensor.matmul(pnh1[:, j, :],
                                     s_bias[0:1, OFF_NH1 + j * 128: OFF_NH1 + (j + 1) * 128],
                                     s_ones[0:1, 0:BC], start=(j == 0), stop=False)
                    for k in range(KT):
                        nc.tensor.matmul(pnh1[:, j, :], wsl(s_whh1, k, m),
                                         hsl(s_h1b, k), start=False,
                                         stop=(j == 7 and k == KT - 1))
                gate_math_stg(pr1, pz1, s_gxs[:, 16:24, usl], pnh1,
                              s_h1f.rearrange("p (k b) -> p k b", b=BC),
                              s_h1b.rearrange("p (k b) -> p k b", b=BC),
                              s_gxs[:, 0:8, usl], s_gxs[:, 8:16, usl])
                plg = psum.tile([128, BC], f32, tag="nx", bufs=1)
                for k in range(KT):
                    nc.tensor.matmul(plg[:], s_embt[:, k * VOCAB:(k + 1) * VOCAB],
                                     hsl(s_h1b, k), start=(k == 0),
                                     stop=(k == KT - 1))
                lsb = stage.tile([128, BC], f32, tag="lg")
                nc.scalar.copy(lsb[:], plg[:])
                nc.sync.dma_start(p_out.ap()[bass.ds(t * VOCAB, VOCAB), :], lsb[:])

        def whole_pipelined():
            # prologue: L0 + gx for the first superstep
            l0_phase(0)
            gx_phase()
            n_loop = t_steps // ss - 1
            if n_loop > 0:
                if unroll >= n_loop:
                    for i in range(n_loop):
                        tb = i * ss
                        l0_phase(tb + ss)
                        l1_phase(tb)
                        gx_phase()
                else:
                    with tc.For_i(0, n_loop * ss, ss,
                                  hint_engines=(mybir.EngineType.PE,)) as tv:
                        l0_phase(tv + ss)
                        l1_phase(tv)
                        gx_phase()
            l1_phase(t_steps - ss)

        def whole_seq():
            n_super = t_steps // ss
            if unroll >= n_super:
                for i in range(n_super):
                    tb = i * ss
                    l0_phase(tb)
                    gx_phase()
                    l1_phase(tb)
            else:
                with tc.For_i(0, t_steps, ss * unroll,
                              hint_engines=(mybir.EngineType.PE,)) as tv:
                    for v in range(unroll):
                        l0_phase(tv + v * ss)
                        gx_phase()
                        l1_phase(tv + v * ss)

        whole = whole_pipelined if pipe else whole_seq
        if ss > 1:
            if reps == 1:
                whole()
            else:
                with tc.For_i(0, reps, 1) as _rep:
                    whole()
        elif unroll >= t_steps:
            for t in range(t_steps):
                step(t)
        elif reps == 1:
            with tc.For_i(0, t_steps, unroll,
                          hint_engines=(mybir.EngineType.PE,)) as tv:
                for u in range(unroll):
                    step(tv + u)
        else:
            with tc.For_i(0, reps, 1) as _rep:
                with tc.For_i(0, t_steps, unroll,
                              hint_engines=(mybir.EngineType.PE,)) as tv:
                    for u in range(unroll):
                        step(tv + u)

    nc.compile()
    return nc


_PROGRAM_CACHE = {}


def _get_program(t_steps, unroll, reps=1, ss=1, pipe=False):
    key = (t_steps, unroll, reps, ss, pipe)
    if key not in _PROGRAM_CACHE:
        _PROGRAM_CACHE[key] = _build_program(t_steps, unroll, reps, ss, pipe)
    return _PROGRAM_CACHE[key]


def run(inputs, t_steps=T, unroll=2, trace=False, ss=1, pipe=False):
    """Run on 8 cores. Returns (full_output (B,T,V) f32, BassKernelResults)."""
    from concourse.bass_utils import run_bass_kernel_spmd

    in_maps = _prep_host(**inputs)
    if t_steps != T:
        for m in in_maps:
            pass  # oh stays full-size; kernel only reads first t_steps slices
    nc = _get_program(t_steps, unroll, 1, ss, pipe)
    res = run_bass_kernel_spmd(nc, in_maps, core_ids=list(range(NCORES)),
                               trace=trace)
    out = np.empty((B, t_steps, VOCAB), np.float32)
    for c in range(NCORES):
        arr = np.asarray(res.results[c]["out"]).reshape(t_steps, VOCAB, BC)
        out[c * BC:(c + 1) * BC] = arr.transpose(2, 0, 1)
    return out, res


def kernel(**inputs) -> np.ndarray:
    out, _ = run(inputs, t_steps=T, unroll=2, trace=False, ss=2)
    return out


def bench(inputs, t_steps=T, unroll=2, iters=3, reps=1, ss=1, pipe=False):
    """Build the sharded jit once, run repeatedly, return (out, times_sec)."""
    import time

    import jax
    import jax.numpy as jnp
    from jax.sharding import Mesh, PartitionSpec
    from jax.experimental.shard_map import shard_map
    import concourse.mybir as mybir
    from concourse import bass2jax
    from concourse.bass2jax import _bass_exec_p, partition_id_tensor

    bass2jax.install_neuronx_cc_hook()
    in_maps = _prep_host(**inputs)
    nc = _get_program(t_steps, unroll, reps, ss, pipe)

    partition_name = nc.partition_id_tensor.name if nc.partition_id_tensor else None
    in_names, out_names, out_avals, zero_outs = [], [], [], []
    for alloc in nc.m.functions[0].allocations:
        if not isinstance(alloc, mybir.MemoryLocationSet):
            continue
        name = alloc.memorylocations[0].name
        if alloc.kind == "ExternalInput":
            if name != partition_name:
                in_names.append(name)
        elif alloc.kind == "ExternalOutput":
            out_names.append(name)
            shape = tuple(alloc.tensor_shape)
            dtype = mybir.dt.np(alloc.dtype)
            out_avals.append(jax.core.ShapedArray(shape, dtype))
            zero_outs.append(np.zeros(shape, dtype))
    n_params = len(in_names)
    n_outs = len(out_avals)
    all_in_names = list(in_names) + list(out_names)
    if partition_name is not None:
        all_in_names.append(partition_name)
    donate = tuple(range(n_params, n_params + n_outs))

    def _body(*args):
        operands = list(args)
        if partition_name is not None:
            operands.append(partition_id_tensor())
        outs = _bass_exec_p.bind(
            *operands,
            out_avals=tuple(out_avals),
            in_names=tuple(all_in_names),
            out_names=tuple(out_names),
            lowering_input_output_aliases=(),
            sim_require_finite=True,
            sim_require_nnan=True,
            nc=nc,
        )
        return tuple(outs)

    devices = jax.devices()[:NCORES]
    mesh = Mesh(np.asarray(devices), ("core",))
    in_specs = (PartitionSpec("core"),) * (n_params + n_outs)
    out_specs = (PartitionSpec("core"),) * len(out_names)
    sharded = jax.jit(
        shard_map(_body, mesh=mesh, in_specs=in_specs, out_specs=out_specs,
                  check_rep=False),
        donate_argnums=donate, keep_unused=True,
    )
    concat_in = [
        np.concatenate([np.asarray(in_maps[c][nm]) for c in range(NCORES)], axis=0)
        for nm in in_names
    ]
    sharding = jax.sharding.NamedSharding(mesh, PartitionSpec("core"))
    dev_in = [jax.device_put(a, sharding) for a in concat_in]

    def zeros():
        return [jax.device_put(
            np.zeros((NCORES * z.shape[0], *z.shape[1:]), z.dtype), sharding)
            for z in zero_outs]

    out_arrs = sharded(*dev_in, *zeros())
    jax.block_until_ready(out_arrs)
    times = []
    for _ in range(iters):
        zs = zeros()
        jax.block_until_ready(zs)
        t0 = time.perf_counter()
        out_arrs2 = sharded(*dev_in, *zs)
        jax.block_until_ready(out_arrs2)
        times.append(time.perf_counter() - t0)
        out_arrs = out_arrs2

    out = np.empty((B, t_steps, VOCAB), np.float32)
    full = np.asarray(out_arrs[0]).reshape(NCORES, t_steps, VOCAB, BC)
    for c in range(NCORES):
        out[c * BC:(c + 1) * BC] = full[c].transpose(2, 0, 1)
    return out, times


# revision 23
# speedup vs baseline: 48.1654x; 1.1452x over previous
"""Trainium2 Bass kernel for a 2-layer GRU phoneme decoder (teacher forcing).

Model: B=512, T=128, H=1024, L=2, V=128.
  tokens = [inp[:,0], target[:,1:]]          (B, T)
  per step t: x = emb[tokens[:,t]]; 2 GRU layers; logits = h1 @ emb.T

Strategy (per the data-parallel sharding hint):
  - Shard batch B=512 across 8 NeuronCores (64 rows each); weights replicated.
  - "Transposed world" layout: feature dims live on SBUF partitions, the
    64-row batch lives on the free axis. All matmuls are weight-stationary:
        out[gates_tile(128), batch(64)] += W_tile(128K,128M).T @ hT(128K,64)
  - Layer-0 input matmul is eliminated: gx0 = onehot(token) @ (emb @ W_ih0.T
    + b_ih0 + b_hh0[rz]) via a 128-row table (vocab on partitions, K=128).
  - Biases folded with K=1 rank-1 matmuls into the PSUM accumulation chains.
  - Gate math fp32 on full-width (128, 512) slabs; h state fp32, cast to bf16
    for the next matmul.
  - All weights stay resident in SBUF for the whole 128-step recurrence.
  - Superstep mode (ss>=2): the layer-1 input matmul gx1 = W_ih1 @ h0 is
    batched over ss consecutive steps (moving width ss*64), staged to SBUF in
    f16, and added to the gh1 PSUM banks during the layer-1 gate math. This
    cuts LDWEIGHTS traffic, the dominant cost: every weight tile must be
    re-streamed into the PE array each step and gets only 64 reuse columns,
    so the kernel sits at the weight-load bandwidth wall (~2 cols/cycle).

  - The h-update tail (sub/mul/add + bf16 cast) runs in 2 k-chunks so the
    next step's matmul chains (which consume h k-tile 0 first) start early.

Measured (differential wall-clock over 30 in-NEFF repetitions, 8 cores):
  v1 (ss=1): ~4.57 ms; ss=2 + chunked tail, unroll=4: ~4.15 ms.
  FLOP roofline ~2.0 ms; weight-reload (LDWEIGHTS) wall for this layout
  ~3.6 ms — every weight tile is re-streamed each of the 128 sequential
  steps with only 64 reuse columns.
"""

import os
import sys

import numpy as np
import ml_dtypes

sys.path.insert(0, "/opt/trn_rl_repo")

VOCAB, H, L, B, T = 128, 1024, 2, 512, 128
NCORES = 8
BC = B // NCORES  # 64 batch rows per core
GH = 3 * H  # 3072 gates
MT = GH // 128  # 24 gate M-tiles
KT = H // 128  # 8 contraction K-tiles
BF = ml_dtypes.bfloat16

# bias vector layout (single row, bf16): [b_hh0_n | b_rz1 | b_ih1_n | b_hh1_n]
OFF_NH0 = 0
OFF_RZ1 = 1024
OFF_NX1 = 1024 + 2048
OFF_NH1 = 1024 + 2048 + 1024
BIAS_LEN = 1024 + 2048 + 1024 + 1024


def _prep_host(inp, target, hidden_state, embedding, W_ih, W_hh, b_ih, b_hh):
    """Pack full inputs into per-core in_maps (all hardware layouts)."""
    inp = np.asarray(inp)
    target = np.asarray(target)
    hidden_state = np.asarray(hidden_state, dtype=np.float32)
    embedding = np.asarray(embedding, dtype=np.float32)
    W_ih = np.asarray(W_ih, dtype=np.float32)
    W_hh = np.asarray(W_hh, dtype=np.float32)
    b_ih = np.asarray(b_ih, dtype=np.float32)
    b_hh = np.asarray(b_hh, dtype=np.float32)

    tokens = np.concatenate([inp[:, :1], target[:, 1:]], axis=1).astype(np.int64)

    def wtiles(W):  # (3072, 1024) -> (128, KT*MT*128) lhsT tiles [p][k,m,c]
        W4 = W.reshape(MT, 128, KT, 128)  # [m, c, k, p]
        return np.ascontiguousarray(
            W4.transpose(3, 2, 0, 1).reshape(128, KT * MT * 128)
        ).astype(BF)

    whh0 = wtiles(W_hh[0])
    wih1 = wtiles(W_ih[1])
    whh1 = wtiles(W_hh[1])

    # layer-0 input table: includes b_ih0 (all gates) and b_hh0 (r,z only)
    tab = embedding @ W_ih[0].T + b_ih[0]
    tab[:, : 2 * H] += b_hh[0][: 2 * H]
    gxtab = np.ascontiguousarray(tab).astype(BF)  # (128, 3072) [v][m*128+c]

    # logits lhsT tiles: [p][k*128+v] = emb[v, k*128+p]
    embt = np.ascontiguousarray(
        embedding.reshape(VOCAB, KT, 128).transpose(2, 1, 0).reshape(128, KT * VOCAB)
    ).astype(BF)

    bias = np.zeros((1, BIAS_LEN), np.float32)
    bias[0, OFF_NH0:OFF_NH0 + H] = b_hh[0][2 * H:]
    bias[0, OFF_RZ1:OFF_RZ1 + 2 * H] = b_ih[1][: 2 * H] + b_hh[1][: 2 * H]
    bias[0, OFF_NX1:OFF_NX1 + H] = b_ih[1][2 * H:]
    bias[0, OFF_NH1:OFF_NH1 + H] = b_hh[1][2 * H:]
    bias = bias.astype(BF)
    ones = np.ones((1, 4 * BC), BF)

    in_maps = []
    for c in range(NCORES):
        sl = slice(c * BC, (c + 1) * BC)
        tk = tokens[sl]  # (BC, T)
        oh = (np.arange(VOCAB)[:, None, None] == tk.T[None, :, :]).astype(BF)
        oh = np.ascontiguousarray(oh.reshape(VOCAB, T * BC))  # [v][t*BC+b]

        def htile(hl):  # (BC, 1024) -> (128, KT*BC) f32  [p][k*BC+b]
            return np.ascontiguousarray(
                hl.reshape(BC, KT, 128).transpose(2, 1, 0).reshape(128, KT * BC)
            ).astype(np.float32)

        in_maps.append(
            {
                "whh0": whh0,
                "wih1": wih1,
                "whh1": whh1,
                "gxtab": gxtab,
                "embt": embt,
                "bias": bias,
                "ones": ones,
                "oh": oh,
                "h0": htile(hidden_state[0, sl]),
                "h1": htile(hidden_state[1, sl]),
            }
        )
    return in_maps


STAGGER = False


def _build_program(t_steps, unroll, reps=1, ss=1, pipe=False):
    import concourse.bass as bass
    import concourse.mybir as mybir
    import concourse.tile as tile
    from concourse import bacc
    from contextlib import ExitStack

    f32 = mybir.dt.float32
    bf16 = mybir.dt.bfloat16
    AF = mybir.ActivationFunctionType

    nc = bacc.Bacc("TRN2", target_bir_lowering=False, debug=False)

    p_whh0 = nc.declare_dram_parameter("whh0", [128, KT * MT * 128], bf16, isOutput=False)
    p_wih1 = nc.declare_dram_parameter("wih1", [128, KT * MT * 128], bf16, isOutput=False)
    p_whh1 = nc.declare_dram_parameter("whh1", [128, KT * MT * 128], bf16, isOutput=False)
    p_gxtab = nc.declare_dram_parameter("gxtab", [128, MT * 128], bf16, isOutput=False)
    p_embt = nc.declare_dram_parameter("embt", [128, KT * VOCAB], bf16, isOutput=False)
    p_bias = nc.declare_dram_parameter("bias", [1, BIAS_LEN], bf16, isOutput=False)
    p_ones = nc.declare_dram_parameter("ones", [1, 4 * BC], bf16, isOutput=False)
    p_oh = nc.declare_dram_parameter("oh", [VOCAB, T * BC], bf16, isOutput=False)
    p_h0 = nc.declare_dram_parameter("h0", [128, KT * BC], f32, isOutput=False)
    p_h1 = nc.declare_dram_parameter("h1", [128, KT * BC], f32, isOutput=False)
    p_out = nc.declare_dram_parameter("out", [t_steps * VOCAB, BC], f32, isOutput=True)

    with ExitStack() as ctx:
        tc = ctx.enter_context(tile.TileContext(nc))
        wpool = ctx.enter_context(tc.tile_pool(name="w", bufs=1))
        state = ctx.enter_context(tc.tile_pool(name="state", bufs=1))
        gates = ctx.enter_context(tc.tile_pool(name="gates", bufs=1))
        stage = ctx.enter_context(tc.tile_pool(name="stage", bufs=3))
        psum = ctx.enter_context(tc.tile_pool(name="psum", bufs=2, space="PSUM"))

        def load(pool, param, shape, dtype, tag):
            t_ = pool.tile(shape, dtype, tag=tag)
            nc.sync.dma_start(t_[:], param.ap())
            return t_

        s_whh0 = load(wpool, p_whh0, [128, KT * MT * 128], bf16, "whh0")
        s_wih1 = load(wpool, p_wih1, [128, KT * MT * 128], bf16, "wih1")
        s_whh1 = load(wpool, p_whh1, [128, KT * MT * 128], bf16, "whh1")
        s_gxtab = load(wpool, p_gxtab, [128, MT * 128], bf16, "gxtab")
        s_embt = load(wpool, p_embt, [128, KT * VOCAB], bf16, "embt")
        s_bias = load(wpool, p_bias, [1, BIAS_LEN], bf16, "bias")
        s_ones = load(wpool, p_ones, [1, 4 * BC], bf16, "ones")
        s_oh = None
        if ss == 1:
            s_oh = load(wpool, p_oh, [VOCAB, T * BC], bf16, "oh_all")
        s_h0f = load(state, p_h0, [128, KT * BC], f32, "h0f")
        s_h1f = load(state, p_h1, [128, KT * BC], f32, "h1f")
        s_h0b = state.tile([128, KT * BC], bf16, tag="h0b")
        s_h1b = state.tile([128, KT * BC], bf16, tag="h1b")
        nc.vector.tensor_copy(s_h0b[:], s_h0f[:])
        nc.vector.tensor_copy(s_h1b[:], s_h1f[:])

        def wsl(s, k, m):  # weight lhsT tile (128, 128)
            o = (k * MT + m) * 128
            return s[:, o:o + 128]

        def hsl(s, k):  # h rhs tile (128, BC)
            return s[:, k * BC:(k + 1) * BC]

        def bank_slice(p, j):
            return p[:, j * BC:(j + 1) * BC]

        def gate_math(pr, pz, pnx, pnh, hf, hb):
            shp = list(pr.shape)
            r_s = gates.tile(shp, f32, tag="rs")
            nc.scalar.activation(r_s[:], pr[:], AF.Sigmoid)
            z_s = gates.tile(shp, f32, tag="zs")
            nc.scalar.activation(z_s[:], pz[:], AF.Sigmoid)
            t1 = gates.tile(shp, f32, tag="tA")
            nc.vector.tensor_mul(t1[:], r_s[:], pnh[:])
            t2 = gates.tile(shp, f32, tag="tB")
            nc.vector.tensor_add(t2[:], pnx[:], t1[:])
            n_s = gates.tile(shp, f32, tag="ns")
            nc.scalar.activation(n_s[:], t2[:], AF.Tanh)
            # h-update + cast in 2 chunks so dependent matmuls (which consume
            # h k-tile 0 first) can start before the full update finishes
            half = shp[-2] // 2 if len(shp) == 3 else None
            for c in range(2):
                if half is not None:
                    cs = (slice(None), slice(c * half, (c + 1) * half), slice(None))
                    cshp = [shp[0], half, shp[2]]
                else:
                    w = shp[-1] // 2
                    cs = (slice(None), slice(c * w, (c + 1) * w))
                    cshp = [shp[0], w]
                d = gates.tile(cshp, f32, tag="tA")
                nc.vector.tensor_sub(d[:], hf[cs], n_s[cs])
                m_ = gates.tile(cshp, f32, tag="tB")
                nc.vector.tensor_mul(m_[:], z_s[cs], d[:])
                nc.vector.tensor_add(hf[cs], n_s[cs], m_[:])
                nc.vector.tensor_copy(hb[cs], hf[cs])

        def bias_mm(pbank, j, boff, start):
            nc.tensor.matmul(
                bank_slice(pbank, j),
                s_bias[0:1, boff + j * 128: boff + (j + 1) * 128],
                s_ones[0:1, 0:BC],
                start=start,
                stop=False,
            )

        def step(t):
            oh_t = stage.tile([128, BC], bf16, tag="oh")
            nc.vector.tensor_copy(oh_t[:], s_oh[:, bass.ds(t * BC, BC)])

            # ---- layer 0 ----
            pr = psum.tile([128, 8 * BC], f32, tag="r")
            pz = psum.tile([128, 8 * BC], f32, tag="z")
            pnx = psum.tile([128, 8 * BC], f32, tag="nx")
            pnh = psum.tile([128, 8 * BC], f32, tag="nh")
            for j in range(8):  # r: table + gh chains
                nc.tensor.matmul(bank_slice(pr, j), s_gxtab[:, j * 128:(j + 1) * 128],
                                 oh_t[:], start=(j == 0), stop=False)
                for k in range(KT):
                    nc.tensor.matmul(bank_slice(pr, j), wsl(s_whh0, k, j),
                                     hsl(s_h0b, k), start=False,
                                     stop=(j == 7 and k == KT - 1))
            for j in range(8):  # z
                m = 8 + j
                nc.tensor.matmul(bank_slice(pz, j), s_gxtab[:, m * 128:(m + 1) * 128],
                                 oh_t[:], start=(j == 0), stop=False)
                for k in range(KT):
                    nc.tensor.matmul(bank_slice(pz, j), wsl(s_whh0, k, m),
                                     hsl(s_h0b, k), start=False,
                                     stop=(j == 7 and k == KT - 1))
            for j in range(8):  # nx: table only (b_ih0 baked in)
                m = 16 + j
                nc.tensor.matmul(bank_slice(pnx, j), s_gxtab[:, m * 128:(m + 1) * 128],
                                 oh_t[:], start=(j == 0), stop=(j == 7))
            for j in range(8):  # nh: bias + gh
                m = 16 + j
                bias_mm(pnh, j, OFF_NH0, start=(j == 0))
                for k in range(KT):
                    nc.tensor.matmul(bank_slice(pnh, j), wsl(s_whh0, k, m),
                                     hsl(s_h0b, k), start=False,
                                     stop=(j == 7 and k == KT - 1))
            gate_math(pr, pz, pnx, pnh, s_h0f, s_h0b)

            # ---- layer 1 ----
            pr1 = psum.tile([128, 8 * BC], f32, tag="r")
            pz1 = psum.tile([128, 8 * BC], f32, tag="z")
            pnx1 = psum.tile([128, 8 * BC], f32, tag="nx")
            pnh1 = psum.tile([128, 8 * BC], f32, tag="nh")
            for j in range(8):  # r = bias + gx(h0) + gh(h1)
                bias_mm(pr1, j, OFF_RZ1, start=(j == 0))
                for k in range(KT):
                    nc.tensor.matmul(bank_slice(pr1, j), wsl(s_wih1, k, j),
                                     hsl(s_h0b, k), start=False, stop=False)
                for k in range(KT):
                    nc.tensor.matmul(bank_slice(pr1, j), wsl(s_whh1, k, j),
                                     hsl(s_h1b, k), start=False,
                                     stop=(j == 7 and k == KT - 1))
            for j in range(8):  # z
                m = 8 + j
                bias_mm(pz1, j, OFF_RZ1 + 1024, start=(j == 0))
                for k in range(KT):
                    nc.tensor.matmul(bank_slice(pz1, j), wsl(s_wih1, k, m),
                                     hsl(s_h0b, k), start=False, stop=False)
                for k in range(KT):
                    nc.tensor.matmul(bank_slice(pz1, j), wsl(s_whh1, k, m),
                                     hsl(s_h1b, k), start=False,
                                     stop=(j == 7 and k == KT - 1))
            for j in range(8):  # nx
                m = 16 + j
                bias_mm(pnx1, j, OFF_NX1, start=(j == 0))
                for k in range(KT):
                    nc.tensor.matmul(bank_slice(pnx1, j), wsl(s_wih1, k, m),
                                     hsl(s_h0b, k), start=False,
                                     stop=(j == 7 and k == KT - 1))
            for j in range(8):  # nh
                m = 16 + j
                bias_mm(pnh1, j, OFF_NH1, start=(j == 0))
                for k in range(KT):
                    nc.tensor.matmul(bank_slice(pnh1, j), wsl(s_whh1, k, m),
                                     hsl(s_h1b, k), start=False,
                                     stop=(j == 7 and k == KT - 1))
            gate_math(pr1, pz1, pnx1, pnh1, s_h1f, s_h1b)

            # ---- logits ----
            plg = psum.tile([128, BC], f32, tag="nx")
            for k in range(KT):
                nc.tensor.matmul(plg[:], s_embt[:, k * VOCAB:(k + 1) * VOCAB],
                                 hsl(s_h1b, k), start=(k == 0), stop=(k == KT - 1))
            lsb = stage.tile([128, BC], f32, tag="lg")
            nc.scalar.copy(lsb[:], plg[:])
            nc.sync.dma_start(p_out.ap()[bass.ds(t * VOCAB, VOCAB), :], lsb[:])

        # ---- superstep (ss >= 2): batch gx1 over ss steps, stage in f16 ----
        f16 = mybir.dt.float16
        SW = ss * BC  # batched moving width
        if ss > 1:
            s_pair = state.tile([128, KT, SW], bf16, tag="h0pair")
            s_gxs = state.tile([128, MT, SW], f16, tag="gxstage")
            CH = 512 // SW  # M-tiles per psum chunk
            NCH = MT // CH

        def l0_step(t, oh_u):
            pr = psum.tile([128, 8, BC], f32, tag="r")
            pz = psum.tile([128, 8, BC], f32, tag="z")
            pnx = psum.tile([128, 8, BC], f32, tag="nx", bufs=1)
            pnh = psum.tile([128, 8, BC], f32, tag="nh", bufs=1)
            for j in range(8):
                nc.tensor.matmul(pr[:, j, :], s_gxtab[:, j * 128:(j + 1) * 128],
                                 oh_u, start=(j == 0), stop=False)
                for k in range(KT):
                    nc.tensor.matmul(pr[:, j, :], wsl(s_whh0, k, j),
                                     hsl(s_h0b, k), start=False,
                                     stop=(j == 7 and k == KT - 1))
            for j in range(8):
                m = 8 + j
                nc.tensor.matmul(pz[:, j, :], s_gxtab[:, m * 128:(m + 1) * 128],
                                 oh_u, start=(j == 0), stop=False)
                for k in range(KT):
                    nc.tensor.matmul(pz[:, j, :], wsl(s_whh0, k, m),
                                     hsl(s_h0b, k), start=False,
                                     stop=(j == 7 and k == KT - 1))
            for j in range(8):
                m = 16 + j
                nc.tensor.matmul(pnx[:, j, :], s_gxtab[:, m * 128:(m + 1) * 128],
                                 oh_u, start=(j == 0), stop=(j == 7))
            for j in range(8):
                m = 16 + j
                nc.tensor.matmul(pnh[:, j, :],
                                 s_bias[0:1, OFF_NH0 + j * 128: OFF_NH0 + (j + 1) * 128],
                                 s_ones[0:1, 0:BC], start=(j == 0), stop=False)
                for k in range(KT):
                    nc.tensor.matmul(pnh[:, j, :], wsl(s_whh0, k, m),
                                     hsl(s_h0b, k), start=False,
                                     stop=(j == 7 and k == KT - 1))
            gate_math(pr, pz, pnx, pnh,
                      s_h0f.rearrange("p (k b) -> p k b", b=BC),
                      s_h0b.rearrange("p (k b) -> p k b", b=BC))

        def gate_math_stg(pr, pz, snx, pnh, hf, hb, sr, sz):
            ar = gates.tile([128, 8, BC], f32, tag="tA")
            nc.vector.tensor_add(ar[:], pr[:], sr)
            az = gates.tile([128, 8, BC], f32, tag="tB")
            nc.vector.tensor_add(az[:], pz[:], sz)
            r_s = gates.tile([128, 8, BC], f32, tag="rs")
            nc.scalar.activation(r_s[:], ar[:], AF.Sigmoid)
            z_s = gates.tile([128, 8, BC], f32, tag="zs")
            nc.scalar.activation(z_s[:], az[:], AF.Sigmoid)
            t1 = gates.tile([128, 8, BC], f32, tag="tA")
            nc.vector.tensor_mul(t1[:], r_s[:], pnh[:])
            t2 = gates.tile([128, 8, BC], f32, tag="tB")
            nc.vector.tensor_add(t2[:], t1[:], snx)
            n_s = gates.tile([128, 8, BC], f32, tag="ns")
            nc.scalar.activation(n_s[:], t2[:], AF.Tanh)
            for c in range(2):
                cs = (slice(None), slice(c * 4, (c + 1) * 4), slice(None))
                d = gates.tile([128, 4, BC], f32, tag="tA")
                nc.vector.tensor_sub(d[:], hf[cs], n_s[cs])
                m_ = gates.tile([128, 4, BC], f32, tag="tB")
                nc.vector.tensor_mul(m_[:], z_s[cs], d[:])
                nc.vector.tensor_add(hf[cs], n_s[cs], m_[:])
                nc.vector.tensor_copy(hb[cs], hf[cs])

        def l0_phase(tb):
            oh_ss = stage.tile([128, SW], bf16, tag="oh")
            nc.sync.dma_start(oh_ss[:], p_oh.ap()[:, bass.ds(tb * BC, SW)])
            hb3 = s_h0b.rearrange("p (k b) -> p k b", b=BC)
            for u in range(ss):
                l0_step(tb + u, oh_ss[:, u * BC:(u + 1) * BC])
                nc.vector.tensor_copy(s_pair[:, :, u * BC:(u + 1) * BC], hb3[:])

        def gx_phase():
            for c in range(NCH):
                pgx = psum.tile([128, CH, SW], f32, tag="gx")
                for mi in range(CH):
                    m = c * CH + mi
                    boff = OFF_RZ1 + m * 128 if m < 16 else OFF_NX1 + (m - 16) * 128
                    nc.tensor.matmul(pgx[:, mi, :],
                                     s_bias[0:1, boff:boff + 128],
                                     s_ones[0:1, 0:SW],
                                     start=(mi == 0), stop=False)
                    for k in range(KT):
                        nc.tensor.matmul(pgx[:, mi, :], wsl(s_wih1, k, m),
                                         s_pair[:, k, :], start=False,
                                         stop=(mi == CH - 1 and k == KT - 1))
                nc.scalar.copy(s_gxs[:, c * CH:(c + 1) * CH, :], pgx[:])

        def l1_phase(tb):
            for u in range(ss):
                t = tb + u
                usl = slice(u * BC, (u + 1) * BC)
                pr1 = psum.tile([128, 8, BC], f32, tag="r")
                pz1 = psum.tile([128, 8, BC], f32, tag="z")
                pnh1 = psum.tile([128, 8, BC], f32, tag="nh", bufs=1)
                for j in range(8):
                    for k in range(KT):
                        nc.tensor.matmul(pr1[:, j, :], wsl(s_whh1, k, j),
                                         hsl(s_h1b, k),
                                         start=(j == 0 and k == 0),
                                         stop=(j == 7 and k == KT - 1))
                for j in range(8):
                    m = 8 + j
                    for k in range(KT):
                        nc.tensor.matmul(pz1[:, j, :], wsl(s_whh1, k, m),
                                         hsl(s_h1b, k),
                                         start=(j == 0 and k == 0),
                                         stop=(j == 7 and k == KT - 1))
                for j in range(8):
                    m = 16 + j
                    nc.tensor.matmul(pnh1[:, j, :],
                                     s_bias[0:1, OFF_NH1 + j * 128: OFF_NH1 + (j + 1) * 128],
                                     s_ones[0:1, 0:BC], start=(j == 0), stop=False)
                    for k in range(KT):
                        nc.tensor.matmul(pnh1[:, j, :], wsl(s_whh1, k, m),
                                         hsl(s_h1b, k), start=False,
                                         stop=(j == 7 and k == KT - 1))
                gate_math_stg(pr1, pz1, s_gxs[:, 16:24, usl], pnh1,
                              s_h1f.rearrange("p (k b) -> p k b", b=BC),
                              s_h1b.rearrange("p (k b) -> p k b", b=BC),
                              s_gxs[:, 0:8, usl], s_gxs[:, 8:16, usl])
                plg = psum.tile([128, BC], f32, tag="nx", bufs=1)
                for k in range(KT):
                    nc.tensor.matmul(plg[:], s_embt[:, k * VOCAB:(k + 1) * VOCAB],
                                     hsl(s_h1b, k), start=(k == 0),
                                     stop=(k == KT - 1))
                lsb = stage.tile([128, BC], f32, tag="lg")
                nc.scalar.copy(lsb[:], plg[:])
                nc.sync.dma_start(p_out.ap()[bass.ds(t * VOCAB, VOCAB), :], lsb[:])

        def whole_pipelined():
            # prologue: L0 + gx for the first superstep
            l0_phase(0)
            gx_phase()
            n_loop = t_steps // ss - 1
            if n_loop > 0:
                if unroll >= n_loop:
                    for i in range(n_loop):
                        tb = i * ss
                        l0_phase(tb + ss)
                        l1_phase(tb)
                        gx_phase()
                else:
                    with tc.For_i(0, n_loop * ss, ss,
                                  hint_engines=(mybir.EngineType.PE,)) as tv:
                        l0_phase(tv + ss)
                        l1_phase(tv)
                        gx_phase()
            l1_phase(t_steps - ss)

        def whole_seq():
            n_super = t_steps // ss
            if unroll >= n_super:
                for i in range(n_super):
                    tb = i * ss
                    l0_phase(tb)
                    gx_phase()
                    l1_phase(tb)
            else:
                with tc.For_i(0, t_steps, ss * unroll,
                              hint_engines=(mybir.EngineType.PE,),
                              staggered_reset=STAGGER) as tv:
                    for v in range(unroll):
                        l0_phase(tv + v * ss)
                        gx_phase()
                        l1_phase(tv + v * ss)

        whole = whole_pipelined if pipe else whole_seq
        if ss > 1:
            if reps == 1:
                whole()
            else:
                with tc.For_i(0, reps, 1) as _rep:
                    whole()
        elif unroll >= t_steps:
            for t in range(t_steps):
                step(t)
        elif reps == 1:
            with tc.For_i(0, t_steps, unroll,
                          hint_engines=(mybir.EngineType.PE,)) as tv:
                for u in range(unroll):
                    step(tv + u)
        else:
            with tc.For_i(0, reps, 1) as _rep:
                with tc.For_i(0, t_steps, unroll,
                              hint_engines=(mybir.EngineType.PE,)) as tv:
                    for u in range(unroll):
                        step(tv + u)

    nc.compile()
    return nc


_PROGRAM_CACHE = {}


def _get_program(t_steps, unroll, reps=1, ss=1, pipe=False):
    key = (t_steps, unroll, reps, ss, pipe)
    if key not in _PROGRAM_CACHE:
        _PROGRAM_CACHE[key] = _build_program(t_steps, unroll, reps, ss, pipe)
    return _PROGRAM_CACHE[key]


def run(inputs, t_steps=T, unroll=2, trace=False, ss=1, pipe=False):
    """Run on 8 cores. Returns (full_output (B,T,V) f32, BassKernelResults)."""
    from concourse.bass_utils import run_bass_kernel_spmd

    in_maps = _prep_host(**inputs)
    if t_steps != T:
        for m in in_maps:
            pass  # oh stays full-size; kernel only reads first t_steps slices
    nc = _get_program(t_steps, unroll, 1, ss, pipe)
    res = run_bass_kernel_spmd(nc, in_maps, core_ids=list(range(NCORES)),
                               trace=trace)
    out = np.empty((B, t_steps, VOCAB), np.float32)
    for c in range(NCORES):
        arr = np.asarray(res.results[c]["out"]).reshape(t_steps, VOCAB, BC)
        out[c * BC:(c + 1) * BC] = arr.transpose(2, 0, 1)
    return out, res


def kernel(**inputs) -> np.ndarray:
    out, _ = run(inputs, t_steps=T, unroll=4, trace=False, ss=2)
    return out


def bench(inputs, t_steps=T, unroll=2, iters=3, reps=1, ss=1, pipe=False):
    """Build the sharded jit once, run repeatedly, return (out, times_sec)."""
    import time

    import jax
    import jax.numpy as jnp
    from jax.sharding import Mesh, PartitionSpec
    from jax.experimental.shard_map import shard_map
    import concourse.mybir as mybir
    from concourse import bass2jax
    from concourse.bass2jax import _bass_exec_p, partition_id_tensor

    bass2jax.install_neuronx_cc_hook()
    in_maps = _prep_host(**inputs)
    nc = _get_program(t_steps, unroll, reps, ss, pipe)

    partition_name = nc.partition_id_tensor.name if nc.partition_id_tensor else None
    in_names, out_names, out_avals, zero_outs = [], [], [], []
    for alloc in nc.m.functions[0].allocations:
        if not isinstance(alloc, mybir.MemoryLocationSet):
            continue
        name = alloc.memorylocations[0].name
        if alloc.kind == "ExternalInput":
            if name != partition_name:
                in_names.append(name)
        elif alloc.kind == "ExternalOutput":
            out_names.append(name)
            shape = tuple(alloc.tensor_shape)
            dtype = mybir.dt.np(alloc.dtype)
            out_avals.append(jax.core.ShapedArray(shape, dtype))
            zero_outs.append(np.zeros(shape, dtype))
    n_params = len(in_names)
    n_outs = len(out_avals)
    all_in_names = list(in_names) + list(out_names)
    if partition_name is not None:
        all_in_names.append(partition_name)
    donate = tuple(range(n_params, n_params + n_outs))

    def _body(*args):
        operands = list(args)
        if partition_name is not None:
            operands.append(partition_id_tensor())
        outs = _bass_exec_p.bind(
            *operands,
            out_avals=tuple(out_avals),
            in_names=tuple(all_in_names),
            out_names=tuple(out_names),
            lowering_input_output_aliases=(),
            sim_require_finite=True,
            sim_require_nnan=True,
            nc=nc,
        )
        return tuple(outs)

    devices = jax.devices()[:NCORES]
    mesh = Mesh(np.asarray(devices), ("core",))
    in_specs = (PartitionSpec("core"),) * (n_params + n_outs)
    out_specs = (PartitionSpec("core"),) * len(out_names)
    sharded = jax.jit(
        shard_map(_body, mesh=mesh, in_specs=in_specs, out_specs=out_specs,
                  check_rep=False),
        donate_argnums=donate, keep_unused=True,
    )
    concat_in = [
        np.concatenate([np.asarray(in_maps[c][nm]) for c in range(NCORES)], axis=0)
        for nm in in_names
    ]
    sharding = jax.sharding.NamedSharding(mesh, PartitionSpec("core"))
    dev_in = [jax.device_put(a, sharding) for a in concat_in]

    def zeros():
        return [jax.device_put(
            np.zeros((NCORES * z.shape[0], *z.shape[1:]), z.dtype), sharding)
            for z in zero_outs]

    out_arrs = sharded(*dev_in, *zeros())
    jax.block_until_ready(out_arrs)
    times = []
    for _ in range(iters):
        zs = zeros()
        jax.block_until_ready(zs)
        t0 = time.perf_counter()
        out_arrs2 = sharded(*dev_in, *zs)
        jax.block_until_ready(out_arrs2)
        times.append(time.perf_counter() - t0)
        out_arrs = out_arrs2

    out = np.empty((B, t_steps, VOCAB), np.float32)
    full = np.asarray(out_arrs[0]).reshape(NCORES, t_steps, VOCAB, BC)
    for c in range(NCORES):
        out[c * BC:(c + 1) * BC] = full[c].transpose(2, 0, 1)
    return out, times
